# revision 67
# baseline (speedup 1.0000x reference)
"""BBox window attention kernel for 8 TRN2 NeuronCores.

Sharding: data-parallel over batch B=8 -> one batch element per core.
Each core computes the full attention for its batch element; no collectives.

Per-core pipeline (all matmuls bf16 with f32 PSUM accumulation):
  1. Phase A streams x in 512-token blocks SHIFTED BY ONE TOKEN (tokens
     1..4096) so windows/v tiles align with block boundaries; token 0 runs
     through a tiny separate path.  Per block: DMA f32 -> DVE cast bf16 ->
     PE transposes (PSUM, drained on ACT) -> xT tile-major feature-major;
     then qkT = W_qk^T @ xT (feature-major q,k) and v = xT^T @ W_v
     (token-major).  Loads lead casts by ~3 blocks, casts lead their
     consumers by a section; dummy ident matmuls warm the PE p-state
     through the DMA startup.
  2. Global token, transposed path: s0T[t,h] via 8-col matmuls, exp ->
     P0T, denominators via ones-matmul (contraction over partitions), o0T
     via v-as-stationary 8-col matmuls, partials accumulated in SBUF.
     Normalization is deferred to the attnT scatter (ACT activation with a
     per-partition scale built by a selector matmul; the selector mask is
     generated with affine_select band conditions).
  3. Windows, 5-stage emission pipeline (S | softmax | P-xbar | slack |
     PV): S matmuls (2 PSUM banks split by head-half) -> exp (ACT,
     unnormalized, both halves into one P tile) -> DVE reduce+recip ->
     Pool broadcast-normalize -> ONE dma_start_transpose of P per
     iteration (SP queue) -> V^T @ P^T -> attnT (feature-major), drained
     on DVE.  Engine roles are kept homogeneous so the Tile scheduler's
     frozen per-engine orders cannot head-of-line-block the chain.
  4. out = attnT^T @ W_out interleaved 2 tiles/iteration as supergroups
     complete; PSUM drained on ACT into 4-tile batches, stores on the SP
     queue deferred one body so dispatches never block on data.
"""

import sys

for _p in ("/opt/trn_rl_repo",):
    if _p not in sys.path:
        sys.path.insert(0, _p)

import numpy as np

import concourse.bass as bass
import concourse.tile as tile
from concourse import bacc, mybir
from concourse.bass_utils import run_bass_kernel_spmd
from concourse.masks import make_identity

F32 = mybir.dt.float32
BF16 = mybir.dt.bfloat16
EXP = mybir.ActivationFunctionType.Exp
IDENT = mybir.ActivationFunctionType.Identity

B, T_FULL, D = 8, 4097, 512
H, WIN, d_head = 8, 64, 64
N_CORES = 8
CH = 4          # head-pair chunks (128 features each)
KC = 4          # contraction chunks of 128 over D
SCALE = float(d_head) ** -0.5
N_WARM = 22
N_WARM2 = 80    # PE p-state warmup matmuls (128-col) covering DMA startup


def _emit(nc, tc, x_d, wqkv_d, wout_d, out_d, T):
    TW = T - 1                  # window tokens (4096)
    NB = TW // 512              # x blocks of 512 tokens
    VT = TW // 128              # v tiles
    WG2 = (TW // WIN) // 16     # supergroups of 16 windows
    TQ = (T + 127) // 128       # output tiles
    assert TW % 512 == 0

    def pool(name, **kw):
        return tc.tile_pool(name=name, **kw)

    with pool("persist", bufs=1) as persist, \
         pool("stats", bufs=4) as stats:

        ident = persist.tile([128, 128], BF16)
        make_identity(nc, ident)

        wqkv_sb = persist.tile([128, KC, 3 * D], BF16)
        wout_sb = persist.tile([128, KC, D], BF16)
        qT = persist.tile([128, CH, T], BF16)
        kT = persist.tile([128, CH, T], BF16)
        v_sb = persist.tile([128, VT, D], BF16)
        v0_sb = persist.tile([1, D], BF16)
        q0all = persist.tile([128, CH, 8], BF16)
        P0T = persist.tile([128, VT, 8], BF16)
        p00 = persist.tile([1, 8], BF16)
        ones_sb = persist.tile([128, 1], BF16)
        o0acc = persist.tile([128, CH, 8], F32)
        s0r = persist.tile([8, 1], F32)

        nc.vector.memset(ones_sb[:, :], 1.0)
        nc.vector.memset(q0all[:, :, :], 0.0)

        # ---- phase A: weights, x load/cast/xbar-transpose, projections ----
        with pool("xstage", bufs=2) as xstage, \
             pool("xTp", bufs=1) as xTp, \
             pool("pA", bufs=8, space="PSUM") as pA:

            # p-state warmup: keep the PE continuously busy from t~1us until
            # the first projection matmuls are ready, so the dispatch-time
            # ramp model reaches full clock before real work arrives.
            warm_ps = pA.tile([128, 128], F32, tag="pa")
            for _ in range(N_WARM):
                nc.tensor.matmul(warm_ps[:, :], ident[:, :], ident[:, :],
                                 start=True, stop=True)

            # xT[p, tt, kc, tc] = x[1 + 128*tt + tc, 128*kc + p]
            xT = xTp.tile([128, NB * 4, KC, 128], BF16)
            s0acc = xstage.tile([8, 1], F32, tag="s0acc", bufs=1)
            nc.vector.memset(s0acc[:, :], 0.0)
            nc.vector.memset(o0acc[:, :, :], 0.0)

            def load_wqkv(hh):
                for kc in range(KC):
                    st = xstage.tile([128, 768], F32, tag="wst", bufs=3)
                    nc.sync.dma_start(
                        out=st[:, :],
                        in_=wqkv_d[128 * kc:128 * kc + 128,
                                   768 * hh:768 * hh + 768],
                    )
                    nc.vector.tensor_copy(
                        wqkv_sb[:, kc, 768 * hh:768 * hh + 768], st[:, :]
                    )

            def load_wout():
                for kc in range(KC):
                    st = xstage.tile([128, 512], F32, tag="wst", bufs=3)
                    nc.sync.dma_start(
                        out=st[:, :], in_=wout_d[128 * kc:128 * kc + 128, :]
                    )
                    nc.vector.tensor_copy(wout_sb[:, kc, :], st[:, :])

            xs_tiles = {}

            def load_block(b):
                xs = xstage.tile([128, 4, D], F32, tag="xs", name="xs", bufs=3)
                nc.sync.dma_start(
                    out=xs[:, :, :],
                    in_=x_d[1 + 512 * b:1 + 512 * b + 512, :].rearrange(
                        "(j p) e -> p j e", p=128),
                )
                xs_tiles[b] = xs

            xc_tiles = {}

            def cast(b):
                # cast on DVE (leads its consumers by a full section)
                xc = xstage.tile([128, 4, D], BF16, tag="xc", name="xc",
                                 bufs=3)
                nc.vector.tensor_copy(xc[:, :, :], xs_tiles.pop(b)[:, :, :])
                xc_tiles[b] = xc

            def tp_drain(b):
                # transpose on PE (self-paced), drain on ACT
                xc = xc_tiles.pop(b)
                for j2 in range(4):
                    tp = pA.tile([128, KC, 128], BF16, tag="pa", name="tp")
                    for kc in range(KC):
                        nc.tensor.transpose(
                            tp[:, kc, :],
                            xc[:, j2, 128 * kc:128 * kc + 128],
                            ident[:, :],
                        )
                    nc.scalar.copy(xT[:, 4 * b + j2, :, :], tp[:, :, :])

            x0_tiles = {}

            def x0_load():
                xs0 = xstage.tile([1, D], F32, tag="xs0", bufs=1)
                nc.sync.dma_start(out=xs0[:, :], in_=x_d[0:1, :])
                xc0 = xstage.tile([1, D], BF16, tag="xc0", bufs=1)
                nc.scalar.copy(xc0[:, :], xs0[:, :])
                x0_tiles["xc0"] = xc0

            def x0_path():
                xc0 = x0_tiles["xc0"]
                tp0 = pA.tile([128, KC, 2], BF16, tag="pa")
                for kc in range(KC):
                    nc.tensor.transpose(
                        tp0[:, kc, 0:1], xc0[:, 128 * kc:128 * kc + 128],
                        ident[0:1, 0:1],
                    )
                xT0 = xstage.tile([128, KC, 1], BF16, tag="xT0", bufs=1)
                nc.vector.tensor_copy(xT0[:, :, :], tp0[:, :, 0:1])
                qk0ps = pA.tile([128, 8], F32, tag="pa")
                for jb in range(8):
                    for kc in range(KC):
                        nc.tensor.matmul(
                            qk0ps[:, jb:jb + 1],
                            wqkv_sb[:, kc, 128 * jb:128 * jb + 128],
                            xT0[:, kc, :],
                            start=(kc == 0), stop=(kc == KC - 1),
                        )
                q0sb = xstage.tile([128, 8], BF16, tag="q0sb", bufs=1)
                nc.vector.tensor_copy(q0sb[:, :], qk0ps[:, :])
                for c in range(CH):
                    nc.vector.tensor_copy(kT[:, c, 0:1], q0sb[:, 4 + c:5 + c])
                for h in range(H):
                    rr = 64 * (h % 2)
                    nc.vector.tensor_copy(
                        q0all[rr:rr + 64, h // 2, h:h + 1],
                        q0sb[rr:rr + 64, h // 2:h // 2 + 1],
                    )
                v0ps = pA.tile([1, D], F32, tag="pa")
                for kc in range(KC):
                    nc.tensor.matmul(
                        v0ps[:, :], xT0[:, kc, :], wqkv_sb[:, kc, 2 * D:3 * D],
                        start=(kc == 0), stop=(kc == KC - 1),
                    )
                nc.vector.tensor_copy(v0_sb[:, :], v0ps[:, :])

            def qkproj(b, jbs):
                c0 = 1 + 512 * b
                for jb in jbs:
                    ps = pA.tile([128, 512], F32, tag="pa")
                    for kc in range(KC):
                        nc.tensor.matmul(
                            ps[:, :],
                            wqkv_sb[:, kc, 128 * jb:128 * jb + 128],
                            xT[:, 4 * b:4 * b + 4, kc, :],
                            start=(kc == 0), stop=(kc == KC - 1),
                        )
                    dst = (qT if jb < 4 else kT)[:, jb % 4, c0:c0 + 512]
                    if jb < 6:
                        nc.vector.tensor_copy(dst, ps[:, :])
                    else:
                        nc.scalar.copy(dst, ps[:, :])

            def vproj(b):
                for j2 in range(4):
                    vt = 4 * b + j2
                    ps = pA.tile([128, D], F32, tag="pa")
                    for kc in range(KC):
                        nc.tensor.matmul(
                            ps[:, :],
                            xT[:, vt, kc, :],
                            wqkv_sb[:, kc, 2 * D:3 * D],
                            start=(kc == 0), stop=(kc == KC - 1),
                        )
                    nc.vector.tensor_copy(v_sb[:, vt, :], ps[:, :])

            def s0t(b):
                # s0T[t, h] for tokens of block b; exp into P0T (unnormalized)
                ps = pA.tile([128, 4, 8], F32, tag="pa")
                for j2 in range(4):
                    vt = 4 * b + j2
                    t0 = 1 + 128 * vt
                    for c in range(CH):
                        nc.tensor.matmul(
                            ps[:, j2, :],
                            kT[:, c, t0:t0 + 128],
                            q0all[:, c, :],
                            start=(c == 0), stop=(c == CH - 1),
                        )
                nc.scalar.activation(
                    P0T[:, 4 * b:4 * b + 4, :].rearrange("p a b -> p (a b)"),
                    ps[:, :, :].rearrange("p a b -> p (a b)"),
                    EXP, bias=0.0, scale=SCALE,
                )

            def sums_o0(b):
                # denominators + o0T contributions for block b (emitted one
                # block late so v/P0T drains are long done); per-block psum
                # partials accumulated into SBUF so no PSUM bank is pinned
                s0p = pA.tile([8, 1], F32, tag="pa", name="s0p")
                o0p = pA.tile([128, CH, 8], F32, tag="pa", name="o0p")
                for j2 in range(4):
                    vt = 4 * b + j2
                    nc.tensor.matmul(
                        s0p[:, :], P0T[:, vt, :], ones_sb[:, :],
                        start=(j2 == 0), stop=(j2 == 3),
                    )
                    for fb in range(CH):
                        nc.tensor.matmul(
                            o0p[:, fb, :],
                            v_sb[:, vt, 128 * fb:128 * fb + 128],
                            P0T[:, vt, :],
                            start=(j2 == 0), stop=(j2 == 3),
                        )
                nc.vector.tensor_tensor(s0acc[:, :], s0acc[:, :], s0p[:, :],
                                        op=mybir.AluOpType.add)
                nc.vector.tensor_tensor(o0acc[:, :, :], o0acc[:, :, :],
                                        o0p[:, :, :],
                                        op=mybir.AluOpType.add)

            # emission order = scheduler priority; DMAs are emitted in true
            # readiness order (loads lead casts/xbars, which lead computes)
            load_block(0)
            cast(0)
            load_wqkv(0)
            for _ in range(N_WARM2):
                nc.tensor.matmul(warm_ps[:, :], ident[:, :], ident[:, :],
                                 start=True, stop=True)
            tp_drain(0)
            load_wqkv(1)
            x0_load()
            load_block(1)
            cast(1)
            load_wout()
            load_block(2)
            for b in range(NB):
                if b + 3 < NB:
                    load_block(b + 3)
                if b + 2 < NB:
                    cast(b + 2)
                qkproj(b, range(0, 4))
                if b + 1 < NB and b > 0:
                    tp_drain(b + 1)
                qkproj(b, range(4, 8))
                if b == 0:
                    tp_drain(1)
                vproj(b)
                if b == 0:
                    x0_path()
                s0t(b)
                if b > 0:
                    sums_o0(b - 1)
            sums_o0(NB - 1)

            # token-0 key column: s00 -> p00; close the accumulation groups
            s00ps = pA.tile([1, 8], F32, tag="pa")
            for c in range(CH):
                nc.tensor.matmul(
                    s00ps[:, :], kT[:, c, 0:1], q0all[:, c, :],
                    start=(c == 0), stop=(c == CH - 1),
                )
            nc.scalar.activation(p00[:, :], s00ps[:, :], EXP,
                                 bias=0.0, scale=SCALE)
            s0p0 = pA.tile([8, 1], F32, tag="pa", name="s0p0")
            o0p0 = pA.tile([128, CH, 8], F32, tag="pa", name="o0p0")
            nc.tensor.matmul(s0p0[:, :], p00[:, :], ones_sb[0:1, :],
                             start=True, stop=True)
            for fb in range(CH):
                nc.tensor.matmul(
                    o0p0[:, fb, :],
                    v0_sb[:, 128 * fb:128 * fb + 128],
                    p00[:, :],
                    start=True, stop=True,
                )
            nc.vector.tensor_tensor(s0acc[:, :], s0acc[:, :], s0p0[:, :],
                                    op=mybir.AluOpType.add)
            nc.vector.tensor_tensor(o0acc[:, :, :], o0acc[:, :, :],
                                    o0p0[:, :, :], op=mybir.AluOpType.add)
            nc.vector.reciprocal(s0r[:, :], s0acc[:, :])

        # ---- windows + output projection ----
        with pool("attnp", bufs=1) as attnp, \
             pool("pp", bufs=4) as ppool, \
             pool("ptp", bufs=4) as ptp, \
             pool("wstats", bufs=4) as wstats, \
             pool("osb", bufs=4) as posb, \
             pool("prow0", bufs=5, space="PSUM") as prow0, \
             pool("prow64", bufs=3, space="PSUM") as prow64:

            attnT = attnp.tile([128, CH, T], BF16)
            selT = attnp.tile([8, CH, 128], F32)
            rep_sb = attnp.tile([128, CH], F32)

            def preamble():
                # scatter o0 into attnT column 0, normalized by 1/s0sum via
                # a per-partition scale vector built by a selector matmul
                # selT[h, c, p] = 1 iff h == 2c + (p >= 64), built with two
                # affine band selects per chunk (partition-aligned accesses)
                nc.gpsimd.memset(selT[:, :, :], 1.0)
                for c in range(CH):
                    nc.gpsimd.affine_select(
                        out=selT[:, c, :], in_=selT[:, c, :],
                        compare_op=mybir.AluOpType.is_ge, fill=0.0,
                        base=63 - 128 * c,
                        pattern=[[-1, 128]], channel_multiplier=64,
                    )
                    nc.gpsimd.affine_select(
                        out=selT[:, c, :], in_=selT[:, c, :],
                        compare_op=mybir.AluOpType.is_ge, fill=0.0,
                        base=128 * c,
                        pattern=[[1, 128]], channel_multiplier=-64,
                    )
                rep_ps = prow0.tile([128, CH], F32, tag="op", bufs=2)
                for c in range(CH):
                    nc.tensor.matmul(rep_ps[:, c:c + 1], selT[:, c, :],
                                     s0r[:, :], start=True, stop=True)
                nc.vector.tensor_copy(rep_sb[:, :], rep_ps[:, :])
                for c in range(CH):
                    nc.scalar.activation(
                        attnT[0:64, c, 0:1], o0acc[0:64, c, 2 * c:2 * c + 1],
                        IDENT, bias=0.0, scale=rep_sb[0:64, c:c + 1])
                    nc.scalar.activation(
                        attnT[64:128, c, 0:1],
                        o0acc[64:128, c, 2 * c + 1:2 * c + 2],
                        IDENT, bias=0.0, scale=rep_sb[64:128, c:c + 1])

            # Window wj (0..15 in a supergroup) maps to (u, b1, s2) =
            # (wj&1, (wj>>1)&1, wj>>2).  Layouts (hardware-validated):
            #   S tile (per head-half r):  [64*b1 + q, slot=2*s2+u, k]
            #   PT (transposed P):         [64*u + k, slab=4*r+s2, 64*b1 + q]
            #   O tile (per parity u):     [64*r + e, slot=2*s2+b1, q]
            def s_stage(wg2, c):
                banks = []
                for r in range(2):
                    sp = (prow0 if r == 0 else prow64).tile(
                        [128, 8, WIN], F32, bufs=2,
                        tag=("S0" if r == 0 else "S1"))
                    for wj in range(16):
                        u, b1, s2 = wj & 1, (wj >> 1) & 1, wj >> 2
                        col0 = 1 + WIN * (16 * wg2 + wj)
                        nc.tensor.matmul(
                            sp[64 * b1:64 * b1 + 64, 2 * s2 + u, :],
                            qT[64 * r:64 * r + 64, c, col0:col0 + WIN],
                            kT[64 * r:64 * r + 64, c, col0:col0 + WIN],
                            start=True, stop=True,
                        )
                    banks.append(sp)
                return banks

            def sm_a(banks, use_dve=False):
                # exp (unnormalized) + sums + recip + Pool normalize.  Both
                # head-half banks land in one P tile so sm_b is a single xbar.
                pb = ppool.tile([128, 2, 8, WIN], BF16, tag="P")
                sums = wstats.tile([128, 2, 8, 1], F32, tag="sums")
                for r in range(2):
                    nc.scalar.activation(
                        pb[:, r, :, :].rearrange("p a b -> p (a b)"),
                        banks[r][:, :, :].rearrange("p a b -> p (a b)"),
                        EXP, bias=0.0, scale=SCALE,
                    )
                    nc.vector.reduce_sum(
                        sums[:, r, :, :], pb[:, r, :, :],
                        axis=mybir.AxisListType.X,
                        op=mybir.AluOpType.add,
                    )
                rs = wstats.tile([128, 2, 8, 1], F32, tag="rs")
                nc.vector.reciprocal(rs[:, :, :, :], sums[:, :, :, :])
                eng = nc.vector if use_dve else nc.gpsimd
                eng.tensor_tensor(
                    pb[:, :, :, :], pb[:, :, :, :],
                    rs[:, :, :, :].broadcast_to([128, 2, 8, WIN]),
                    op=mybir.AluOpType.mult,
                )
                return pb

            def sm_b(pb):
                PT_sb = ptp.tile([128, 8, 128], BF16, tag="PT")
                nc.sync.dma_start_transpose(
                    out=PT_sb[:, :, :], in_=pb[:, :, :, :]
                )
                return PT_sb

            def bk_stage(wg2, c, PT_sb):
                cb = 1 + 1024 * wg2
                av = attnT[:, c, cb:cb + 1024].rearrange(
                    "p (a b u q) -> p a b u q", a=4, b=2, u=2)
                for u in range(2):
                    op = (prow0 if u == 0 else prow64).tile(
                        [128, 8, WIN], F32, bufs=1,
                        tag=("O0" if u == 0 else "O1"))
                    for b1 in range(2):
                        for s2 in range(4):
                            wp = 8 * wg2 + 2 * s2 + b1
                            for r in range(2):
                                h = 2 * c + r
                                nc.tensor.matmul(
                                    op[64 * r:64 * r + 64, 2 * s2 + b1, :],
                                    v_sb[64 * u:64 * u + 64, wp,
                                         64 * h:64 * h + 64],
                                    PT_sb[64 * u:64 * u + 64, 4 * r + s2,
                                          64 * b1:64 * b1 + 64],
                                    start=True, stop=True,
                                )
                    nc.vector.tensor_copy(
                        av[:, :, :, u, :],
                        op[:, :, :].rearrange("p (a b) q -> p a b q", a=4),
                    )

            ob_state = {}
            OBN = 4
            pending_stores = []

            def flush_stores():
                # store dispatches deferred a body so the SP queue never
                # blocks on drain data (SP also carries the PT xbars)
                for rr, nrows, ob in pending_stores:
                    full, tail = nrows // 128, nrows % 128
                    if full:
                        nc.sync.dma_start(
                            out=out_d[rr:rr + 128 * full, :].rearrange(
                                "(j p) e -> p j e", p=128),
                            in_=ob[:, 0:full, :],
                        )
                    if tail:
                        nc.sync.dma_start(
                            out=out_d[rr + 128 * full:rr + 128 * full + tail,
                                      :],
                            in_=ob[:tail, full, :])
                del pending_stores[:]

            def outproj(tq):
                r0 = 128 * tq
                rows = min(128, T - r0)
                ps = prow0.tile([128, D], F32, tag="op", bufs=2)
                for c in range(CH):
                    nc.tensor.matmul(
                        ps[:rows, :],
                        attnT[:, c, r0:r0 + rows],
                        wout_sb[:, c, :],
                        start=(c == 0), stop=(c == CH - 1),
                    )
                # drains on ACT (latency-tolerant); DVE keeps the softmax path
                if tq % OBN == 0:
                    ob_state["t"] = posb.tile([128, OBN, D], F32, tag="ob",
                                              name="ob4", bufs=2)
                ob2 = ob_state["t"]
                if tq >= 24 and tq % 2 == 1:
                    nc.vector.tensor_copy(ob2[:rows, tq % OBN, :],
                                          ps[:rows, :])
                else:
                    nc.scalar.copy(ob2[:rows, tq % OBN, :], ps[:rows, :])
                if tq % OBN == OBN - 1 or tq == TQ - 1:
                    base = tq - tq % OBN
                    pending_stores.append((128 * base,
                                           128 * (tq % OBN) + rows, ob2))

            # 5-stage pipeline: S(j) | sm_a(j-1) | sm_b(j-2) | slack | bk(j-4)
            its = [(wg2, c) for wg2 in range(WG2) for c in range(CH)]
            NIT = len(its)
            stage_s, stage_p, stage_t = {}, {}, {}
            state = {"done": 0, "ready": 0}

            def op_some(nmax):
                while state["done"] < state["ready"] and nmax > 0:
                    outproj(state["done"])
                    state["done"] += 1
                    nmax -= 1

            ready_updates = []
            for j in range(NIT + 4):
                # outproj first: its PSUM is drained early in the body so the
                # ACT drain never gates this body's exp chain.  Tiles become
                # eligible two bodies after their supergroup's last BK so the
                # attnT drains are never chased.
                flush_stores()
                for (eb, rv) in list(ready_updates):
                    if j >= eb:
                        state["ready"] = max(state["ready"], rv)
                        ready_updates.remove((eb, rv))
                op_some(2 if j < NIT else 3)
                if j < NIT:
                    stage_s[j] = s_stage(*its[j])
                    stage_p[j] = sm_a(stage_s.pop(j), use_dve=(j >= NIT - 2))
                if j == 3:
                    preamble()
                if 0 <= j - 2 < NIT:
                    stage_t[j - 2] = sm_b(stage_p.pop(j - 2))
                if 0 <= j - 4 < NIT:
                    i = j - 4
                    bit = its[i]
                    bk_stage(bit[0], bit[1], stage_t.pop(i))
                    if bit[1] == CH - 1:
                        rv = TQ if bit[0] == WG2 - 1 else 8 * (bit[0] + 1)
                        ready_updates.append((j + 1, rv))
            state["ready"] = TQ
            op_some(TQ)
            flush_stores()


def build(T=T_FULL):
    nc = bacc.Bacc("TRN2", target_bir_lowering=False, debug=False,
                   num_devices=N_CORES)
    x_d = nc.dram_tensor("x", [T, D], F32, kind="ExternalInput")
    wqkv_d = nc.dram_tensor("w_qkv", [D, 3 * D], F32, kind="ExternalInput")
    wout_d = nc.dram_tensor("w_out", [D, D], F32, kind="ExternalInput")
    out_d = nc.dram_tensor("out", [T, D], F32, kind="ExternalOutput")
    with tile.TileContext(nc) as tc:
        _emit(nc, tc, x_d.ap(), wqkv_d.ap(), wout_d.ap(), out_d.ap(), T)
    nc.compile()
    return nc


_NC_CACHE = {}


def kernel(x, w_qkv, w_out):
    x = np.ascontiguousarray(np.asarray(x, dtype=np.float32))
    w_qkv = np.ascontiguousarray(np.asarray(w_qkv, dtype=np.float32))
    w_out = np.ascontiguousarray(np.asarray(w_out, dtype=np.float32))
    assert x.shape == (B, T_FULL, D)

    if "nc" not in _NC_CACHE:
        _NC_CACHE["nc"] = build(T_FULL)
    nc = _NC_CACHE["nc"]

    in_maps = [
        {"x": x[b], "w_qkv": w_qkv, "w_out": w_out} for b in range(N_CORES)
    ]
    last_err = None
    for _attempt in range(4):
        try:
            res = run_bass_kernel_spmd(nc, in_maps, core_ids=list(range(N_CORES)))
            break
        except Exception as e:  # transient NRT device errors
            last_err = e
            try:  # force a fresh PJRT client before retrying
                import jax
                jax.clear_caches()
                jax.extend.backend.clear_backends()
            except Exception:
                pass
            import time as _time
            _time.sleep(5)
    else:
        raise last_err
    return np.stack([res.results[b]["out"] for b in range(N_CORES)], axis=0)


# revision 74
# speedup vs baseline: 1.0185x; 1.0185x over previous
"""BBox window attention kernel for 8 TRN2 NeuronCores.

Sharding: data-parallel over batch B=8 -> one batch element per core.
Each core computes the full attention for its batch element; no collectives.

Per-core pipeline (all matmuls bf16 with f32 PSUM accumulation):
  1. Phase A streams x in 512-token blocks SHIFTED BY ONE TOKEN (tokens
     1..4096) so windows/v tiles align with block boundaries; token 0 runs
     through a tiny separate path.  Per block: DMA f32 -> DVE cast bf16 ->
     PE transposes (PSUM, drained on ACT) -> xT tile-major feature-major;
     then qkT = W_qk^T @ xT (feature-major q,k) and v = xT^T @ W_v
     (token-major).  Loads lead casts by ~3 blocks, casts lead their
     consumers by a section; dummy ident matmuls warm the PE p-state
     through the DMA startup.
  2. Global token, transposed path: s0T[t,h] via 8-col matmuls, exp ->
     P0T, denominators via ones-matmul (contraction over partitions), o0T
     via v-as-stationary 8-col matmuls, partials accumulated in SBUF.
     Normalization is deferred to the attnT scatter (ACT activation with a
     per-partition scale built by a selector matmul; the selector mask is
     generated with affine_select band conditions).
  3. Windows, 5-stage emission pipeline (S | softmax | P-xbar | slack |
     PV): S matmuls (2 PSUM banks split by head-half) -> exp (ACT,
     unnormalized, both halves into one P tile) -> DVE reduce+recip ->
     Pool broadcast-normalize -> ONE dma_start_transpose of P per
     iteration (SP queue) -> V^T @ P^T -> attnT (feature-major), drained
     on DVE.  Engine roles are kept homogeneous so the Tile scheduler's
     frozen per-engine orders cannot head-of-line-block the chain.
  4. out = attnT^T @ W_out interleaved 2 tiles/iteration as supergroups
     complete; PSUM drained on ACT into 4-tile batches, stores on the SP
     queue deferred one body so dispatches never block on data.
"""

import sys

for _p in ("/opt/trn_rl_repo",):
    if _p not in sys.path:
        sys.path.insert(0, _p)

import numpy as np

import concourse.bass as bass
import concourse.tile as tile
from concourse import bacc, mybir
from concourse.bass_utils import run_bass_kernel_spmd
from concourse.masks import make_identity

F32 = mybir.dt.float32
BF16 = mybir.dt.bfloat16
EXP = mybir.ActivationFunctionType.Exp
IDENT = mybir.ActivationFunctionType.Identity

B, T_FULL, D = 8, 4097, 512
H, WIN, d_head = 8, 64, 64
N_CORES = 8
CH = 4          # head-pair chunks (128 features each)
KC = 4          # contraction chunks of 128 over D
SCALE = float(d_head) ** -0.5
N_WARM = 22
N_WARM2 = 80    # PE p-state warmup matmuls (128-col) covering DMA startup


def _emit(nc, tc, x_d, wqkv_d, wout_d, out_d, T):
    TW = T - 1                  # window tokens (4096)
    NB = TW // 512              # x blocks of 512 tokens
    VT = TW // 128              # v tiles
    WG2 = (TW // WIN) // 16     # supergroups of 16 windows
    TQ = (T + 127) // 128       # output tiles
    assert TW % 512 == 0

    def pool(name, **kw):
        return tc.tile_pool(name=name, **kw)

    with pool("persist", bufs=1) as persist, \
         pool("stats", bufs=4) as stats:

        ident = persist.tile([128, 128], BF16)
        make_identity(nc, ident)

        wqkv_sb = persist.tile([128, KC, 3 * D], BF16)
        wout_sb = persist.tile([128, KC, D], BF16)
        qT = persist.tile([128, CH, T], BF16)
        kT = persist.tile([128, CH, T], BF16)
        v_sb = persist.tile([128, VT, D], BF16)
        v0_sb = persist.tile([1, D], BF16)
        q0all = persist.tile([128, CH, 8], BF16)
        P0T = persist.tile([128, VT, 8], BF16)
        p00 = persist.tile([1, 8], BF16)
        ones_sb = persist.tile([128, 1], BF16)
        o0acc = persist.tile([128, CH, 8], F32)
        s0r = persist.tile([8, 1], F32)

        nc.vector.memset(ones_sb[:, :], 1.0)
        nc.vector.memset(q0all[:, :, :], 0.0)

        # ---- phase A: weights, x load/cast/xbar-transpose, projections ----
        with pool("xstage", bufs=2) as xstage, \
             pool("xTp", bufs=1) as xTp, \
             pool("pA", bufs=8, space="PSUM") as pA:

            # p-state warmup: keep the PE continuously busy from t~1us until
            # the first projection matmuls are ready, so the dispatch-time
            # ramp model reaches full clock before real work arrives.
            warm_ps = pA.tile([128, 128], F32, tag="pa")
            for _ in range(N_WARM):
                nc.tensor.matmul(warm_ps[:, :], ident[:, :], ident[:, :],
                                 start=True, stop=True)

            # xT[p, tt, kc, tc] = x[1 + 128*tt + tc, 128*kc + p]
            xT = xTp.tile([128, NB * 4, KC, 128], BF16)
            s0acc = xstage.tile([8, 1], F32, tag="s0acc", bufs=1)
            nc.vector.memset(s0acc[:, :], 0.0)
            nc.vector.memset(o0acc[:, :, :], 0.0)

            def load_wqkv(hh):
                for kc in range(KC):
                    st = xstage.tile([128, 768], F32, tag="wst", bufs=3)
                    nc.sync.dma_start(
                        out=st[:, :],
                        in_=wqkv_d[128 * kc:128 * kc + 128,
                                   768 * hh:768 * hh + 768],
                    )
                    nc.vector.tensor_copy(
                        wqkv_sb[:, kc, 768 * hh:768 * hh + 768], st[:, :]
                    )

            def load_wout():
                for kc in range(KC):
                    st = xstage.tile([128, 512], F32, tag="wst", bufs=3)
                    nc.sync.dma_start(
                        out=st[:, :], in_=wout_d[128 * kc:128 * kc + 128, :]
                    )
                    nc.vector.tensor_copy(wout_sb[:, kc, :], st[:, :])

            xs_tiles = {}

            def load_block(b):
                xs = xstage.tile([128, 4, D], F32, tag="xs", name="xs", bufs=3)
                nc.sync.dma_start(
                    out=xs[:, :, :],
                    in_=x_d[1 + 512 * b:1 + 512 * b + 512, :].rearrange(
                        "(j p) e -> p j e", p=128),
                )
                xs_tiles[b] = xs

            xc_tiles = {}

            def cast(b):
                # cast on DVE (leads its consumers by a full section)
                xc = xstage.tile([128, 4, D], BF16, tag="xc", name="xc",
                                 bufs=3)
                nc.vector.tensor_copy(xc[:, :, :], xs_tiles.pop(b)[:, :, :])
                xc_tiles[b] = xc

            def tp_drain(b):
                # transpose on PE (self-paced), drain on ACT
                xc = xc_tiles.pop(b)
                for j2 in range(4):
                    tp = pA.tile([128, KC, 128], BF16, tag="pa", name="tp")
                    for kc in range(KC):
                        nc.tensor.transpose(
                            tp[:, kc, :],
                            xc[:, j2, 128 * kc:128 * kc + 128],
                            ident[:, :],
                        )
                    nc.scalar.copy(xT[:, 4 * b + j2, :, :], tp[:, :, :])

            x0_tiles = {}

            def x0_load():
                xs0 = xstage.tile([1, D], F32, tag="xs0", bufs=1)
                nc.sync.dma_start(out=xs0[:, :], in_=x_d[0:1, :])
                xc0 = xstage.tile([1, D], BF16, tag="xc0", bufs=1)
                nc.scalar.copy(xc0[:, :], xs0[:, :])
                x0_tiles["xc0"] = xc0

            def x0_path():
                xc0 = x0_tiles["xc0"]
                tp0 = pA.tile([128, KC, 2], BF16, tag="pa")
                for kc in range(KC):
                    nc.tensor.transpose(
                        tp0[:, kc, 0:1], xc0[:, 128 * kc:128 * kc + 128],
                        ident[0:1, 0:1],
                    )
                xT0 = xstage.tile([128, KC, 1], BF16, tag="xT0", bufs=1)
                nc.vector.tensor_copy(xT0[:, :, :], tp0[:, :, 0:1])
                qk0ps = pA.tile([128, 8], F32, tag="pa")
                for jb in range(8):
                    for kc in range(KC):
                        nc.tensor.matmul(
                            qk0ps[:, jb:jb + 1],
                            wqkv_sb[:, kc, 128 * jb:128 * jb + 128],
                            xT0[:, kc, :],
                            start=(kc == 0), stop=(kc == KC - 1),
                        )
                q0sb = xstage.tile([128, 8], BF16, tag="q0sb", bufs=1)
                nc.vector.tensor_copy(q0sb[:, :], qk0ps[:, :])
                for c in range(CH):
                    nc.vector.tensor_copy(kT[:, c, 0:1], q0sb[:, 4 + c:5 + c])
                for h in range(H):
                    rr = 64 * (h % 2)
                    nc.vector.tensor_copy(
                        q0all[rr:rr + 64, h // 2, h:h + 1],
                        q0sb[rr:rr + 64, h // 2:h // 2 + 1],
                    )
                v0ps = pA.tile([1, D], F32, tag="pa")
                for kc in range(KC):
                    nc.tensor.matmul(
                        v0ps[:, :], xT0[:, kc, :], wqkv_sb[:, kc, 2 * D:3 * D],
                        start=(kc == 0), stop=(kc == KC - 1),
                    )
                nc.vector.tensor_copy(v0_sb[:, :], v0ps[:, :])

            def qkproj(b, jbs):
                c0 = 1 + 512 * b
                for jb in jbs:
                    ps = pA.tile([128, 512], F32, tag="pa")
                    for kc in range(KC):
                        nc.tensor.matmul(
                            ps[:, :],
                            wqkv_sb[:, kc, 128 * jb:128 * jb + 128],
                            xT[:, 4 * b:4 * b + 4, kc, :],
                            start=(kc == 0), stop=(kc == KC - 1),
                        )
                    dst = (qT if jb < 4 else kT)[:, jb % 4, c0:c0 + 512]
                    if jb < 6:
                        nc.vector.tensor_copy(dst, ps[:, :])
                    else:
                        nc.scalar.copy(dst, ps[:, :])

            def vproj(b):
                for j2 in range(4):
                    vt = 4 * b + j2
                    ps = pA.tile([128, D], F32, tag="pa")
                    for kc in range(KC):
                        nc.tensor.matmul(
                            ps[:, :],
                            xT[:, vt, kc, :],
                            wqkv_sb[:, kc, 2 * D:3 * D],
                            start=(kc == 0), stop=(kc == KC - 1),
                        )
                    nc.vector.tensor_copy(v_sb[:, vt, :], ps[:, :])

            def s0t(b):
                # s0T[t, h] for tokens of block b; exp into P0T (unnormalized)
                ps = pA.tile([128, 4, 8], F32, tag="pa")
                for j2 in range(4):
                    vt = 4 * b + j2
                    t0 = 1 + 128 * vt
                    for c in range(CH):
                        nc.tensor.matmul(
                            ps[:, j2, :],
                            kT[:, c, t0:t0 + 128],
                            q0all[:, c, :],
                            start=(c == 0), stop=(c == CH - 1),
                        )
                nc.scalar.activation(
                    P0T[:, 4 * b:4 * b + 4, :].rearrange("p a b -> p (a b)"),
                    ps[:, :, :].rearrange("p a b -> p (a b)"),
                    EXP, bias=0.0, scale=SCALE,
                )

            def sums_o0(b):
                # denominators + o0T contributions for block b (emitted one
                # block late so v/P0T drains are long done); per-block psum
                # partials accumulated into SBUF so no PSUM bank is pinned
                s0p = pA.tile([8, 1], F32, tag="pa", name="s0p")
                o0p = pA.tile([128, CH, 8], F32, tag="pa", name="o0p")
                for j2 in range(4):
                    vt = 4 * b + j2
                    nc.tensor.matmul(
                        s0p[:, :], P0T[:, vt, :], ones_sb[:, :],
                        start=(j2 == 0), stop=(j2 == 3),
                    )
                    for fb in range(CH):
                        nc.tensor.matmul(
                            o0p[:, fb, :],
                            v_sb[:, vt, 128 * fb:128 * fb + 128],
                            P0T[:, vt, :],
                            start=(j2 == 0), stop=(j2 == 3),
                        )
                nc.vector.tensor_tensor(s0acc[:, :], s0acc[:, :], s0p[:, :],
                                        op=mybir.AluOpType.add)
                nc.vector.tensor_tensor(o0acc[:, :, :], o0acc[:, :, :],
                                        o0p[:, :, :],
                                        op=mybir.AluOpType.add)

            # emission order = scheduler priority; DMAs are emitted in true
            # readiness order (loads lead casts/xbars, which lead computes)
            load_block(0)
            cast(0)
            load_wqkv(0)
            for _ in range(N_WARM2):
                nc.tensor.matmul(warm_ps[:, :], ident[:, :], ident[:, :],
                                 start=True, stop=True)
            tp_drain(0)
            load_wqkv(1)
            x0_load()
            load_block(1)
            cast(1)
            load_wout()
            load_block(2)
            for b in range(NB):
                if b + 3 < NB:
                    load_block(b + 3)
                if b + 2 < NB:
                    cast(b + 2)
                qkproj(b, range(0, 4))
                if b + 1 < NB and b > 0:
                    tp_drain(b + 1)
                qkproj(b, range(4, 8))
                if b == 0:
                    tp_drain(1)
                vproj(b)
                if b == 0:
                    x0_path()
                s0t(b)
                if b > 0:
                    sums_o0(b - 1)
            sums_o0(NB - 1)

            # token-0 key column: s00 -> p00; close the accumulation groups
            s00ps = pA.tile([1, 8], F32, tag="pa")
            for c in range(CH):
                nc.tensor.matmul(
                    s00ps[:, :], kT[:, c, 0:1], q0all[:, c, :],
                    start=(c == 0), stop=(c == CH - 1),
                )
            nc.scalar.activation(p00[:, :], s00ps[:, :], EXP,
                                 bias=0.0, scale=SCALE)
            s0p0 = pA.tile([8, 1], F32, tag="pa", name="s0p0")
            o0p0 = pA.tile([128, CH, 8], F32, tag="pa", name="o0p0")
            nc.tensor.matmul(s0p0[:, :], p00[:, :], ones_sb[0:1, :],
                             start=True, stop=True)
            for fb in range(CH):
                nc.tensor.matmul(
                    o0p0[:, fb, :],
                    v0_sb[:, 128 * fb:128 * fb + 128],
                    p00[:, :],
                    start=True, stop=True,
                )
            nc.vector.tensor_tensor(s0acc[:, :], s0acc[:, :], s0p0[:, :],
                                    op=mybir.AluOpType.add)
            nc.vector.tensor_tensor(o0acc[:, :, :], o0acc[:, :, :],
                                    o0p0[:, :, :], op=mybir.AluOpType.add)
            nc.vector.reciprocal(s0r[:, :], s0acc[:, :])

        # ---- windows + output projection ----
        with pool("attnp", bufs=1) as attnp, \
             pool("pp", bufs=4) as ppool, \
             pool("ptp", bufs=4) as ptp, \
             pool("wstats", bufs=4) as wstats, \
             pool("osb", bufs=4) as posb, \
             pool("prow0", bufs=5, space="PSUM") as prow0, \
             pool("prow64", bufs=3, space="PSUM") as prow64:

            attnT = attnp.tile([128, CH, T], BF16)
            selT = attnp.tile([8, CH, 128], F32)
            rep_sb = attnp.tile([128, CH], F32)

            def preamble():
                # scatter o0 into attnT column 0, normalized by 1/s0sum via
                # a per-partition scale vector built by a selector matmul
                # selT[h, c, p] = 1 iff h == 2c + (p >= 64), built with two
                # affine band selects per chunk (partition-aligned accesses)
                nc.gpsimd.memset(selT[:, :, :], 1.0)
                for c in range(CH):
                    nc.gpsimd.affine_select(
                        out=selT[:, c, :], in_=selT[:, c, :],
                        compare_op=mybir.AluOpType.is_ge, fill=0.0,
                        base=63 - 128 * c,
                        pattern=[[-1, 128]], channel_multiplier=64,
                    )
                    nc.gpsimd.affine_select(
                        out=selT[:, c, :], in_=selT[:, c, :],
                        compare_op=mybir.AluOpType.is_ge, fill=0.0,
                        base=128 * c,
                        pattern=[[1, 128]], channel_multiplier=-64,
                    )
                rep_ps = prow0.tile([128, CH], F32, tag="op", bufs=2)
                for c in range(CH):
                    nc.tensor.matmul(rep_ps[:, c:c + 1], selT[:, c, :],
                                     s0r[:, :], start=True, stop=True)
                nc.vector.tensor_copy(rep_sb[:, :], rep_ps[:, :])
                for c in range(CH):
                    nc.scalar.activation(
                        attnT[0:64, c, 0:1], o0acc[0:64, c, 2 * c:2 * c + 1],
                        IDENT, bias=0.0, scale=rep_sb[0:64, c:c + 1])
                    nc.scalar.activation(
                        attnT[64:128, c, 0:1],
                        o0acc[64:128, c, 2 * c + 1:2 * c + 2],
                        IDENT, bias=0.0, scale=rep_sb[64:128, c:c + 1])

            # Window wj (0..15 in a supergroup) maps to (u, b1, s2) =
            # (wj&1, (wj>>1)&1, wj>>2).  Layouts (hardware-validated):
            #   S tile (per head-half r):  [64*b1 + q, slot=2*s2+u, k]
            #   PT (transposed P):         [64*u + k, slab=4*r+s2, 64*b1 + q]
            #   O tile (per parity u):     [64*r + e, slot=2*s2+b1, q]
            def s_stage(wg2, c):
                banks = []
                for r in range(2):
                    sp = (prow0 if r == 0 else prow64).tile(
                        [128, 8, WIN], F32, bufs=2,
                        tag=("S0" if r == 0 else "S1"))
                    for wj in range(16):
                        u, b1, s2 = wj & 1, (wj >> 1) & 1, wj >> 2
                        col0 = 1 + WIN * (16 * wg2 + wj)
                        nc.tensor.matmul(
                            sp[64 * b1:64 * b1 + 64, 2 * s2 + u, :],
                            qT[64 * r:64 * r + 64, c, col0:col0 + WIN],
                            kT[64 * r:64 * r + 64, c, col0:col0 + WIN],
                            start=True, stop=True,
                        )
                    banks.append(sp)
                return banks

            def sm_a(banks, use_dve=False):
                # exp (unnormalized) + sums + recip + Pool normalize.  Both
                # head-half banks land in one P tile so sm_b is a single xbar.
                pb = ppool.tile([128, 2, 8, WIN], BF16, tag="P")
                sums = wstats.tile([128, 2, 8, 1], F32, tag="sums")
                for r in range(2):
                    nc.scalar.activation(
                        pb[:, r, :, :].rearrange("p a b -> p (a b)"),
                        banks[r][:, :, :].rearrange("p a b -> p (a b)"),
                        EXP, bias=0.0, scale=SCALE,
                    )
                    nc.vector.reduce_sum(
                        sums[:, r, :, :], pb[:, r, :, :],
                        axis=mybir.AxisListType.X,
                        op=mybir.AluOpType.add,
                    )
                rs = wstats.tile([128, 2, 8, 1], F32, tag="rs")
                nc.vector.reciprocal(rs[:, :, :, :], sums[:, :, :, :])
                eng = nc.vector if use_dve else nc.gpsimd
                eng.tensor_tensor(
                    pb[:, :, :, :], pb[:, :, :, :],
                    rs[:, :, :, :].broadcast_to([128, 2, 8, WIN]),
                    op=mybir.AluOpType.mult,
                )
                return pb

            def sm_b(pb):
                PT_sb = ptp.tile([128, 8, 128], BF16, tag="PT")
                nc.sync.dma_start_transpose(
                    out=PT_sb[:, :, :], in_=pb[:, :, :, :]
                )
                return PT_sb

            def bk_stage(wg2, c, PT_sb):
                cb = 1 + 1024 * wg2
                av = attnT[:, c, cb:cb + 1024].rearrange(
                    "p (a b u q) -> p a b u q", a=4, b=2, u=2)
                for u in range(2):
                    op = (prow0 if u == 0 else prow64).tile(
                        [128, 8, WIN], F32, bufs=1,
                        tag=("O0" if u == 0 else "O1"))
                    for b1 in range(2):
                        for s2 in range(4):
                            wp = 8 * wg2 + 2 * s2 + b1
                            for r in range(2):
                                h = 2 * c + r
                                nc.tensor.matmul(
                                    op[64 * r:64 * r + 64, 2 * s2 + b1, :],
                                    v_sb[64 * u:64 * u + 64, wp,
                                         64 * h:64 * h + 64],
                                    PT_sb[64 * u:64 * u + 64, 4 * r + s2,
                                          64 * b1:64 * b1 + 64],
                                    start=True, stop=True,
                                )
                    nc.vector.tensor_copy(
                        av[:, :, :, u, :],
                        op[:, :, :].rearrange("p (a b) q -> p a b q", a=4),
                    )

            ob_state = {}
            OBN = 8
            pending_stores = []

            def flush_stores():
                # store dispatches deferred a body so the SP queue never
                # blocks on drain data (SP also carries the PT xbars)
                for rr, nrows, ob in pending_stores:
                    full, tail = nrows // 128, nrows % 128
                    if full:
                        nc.sync.dma_start(
                            out=out_d[rr:rr + 128 * full, :].rearrange(
                                "(j p) e -> p j e", p=128),
                            in_=ob[:, 0:full, :],
                        )
                    if tail:
                        nc.sync.dma_start(
                            out=out_d[rr + 128 * full:rr + 128 * full + tail,
                                      :],
                            in_=ob[:tail, full, :])
                del pending_stores[:]

            def outproj(tq):
                r0 = 128 * tq
                rows = min(128, T - r0)
                ps = prow0.tile([128, D], F32, tag="op", bufs=2)
                for c in range(CH):
                    nc.tensor.matmul(
                        ps[:rows, :],
                        attnT[:, c, r0:r0 + rows],
                        wout_sb[:, c, :],
                        start=(c == 0), stop=(c == CH - 1),
                    )
                # drains on ACT (latency-tolerant); DVE keeps the softmax path
                if tq % OBN == 0:
                    ob_state["t"] = posb.tile([128, OBN, D], F32, tag="ob",
                                              name="ob4", bufs=2)
                ob2 = ob_state["t"]
                if tq >= 24 and tq % 2 == 1:
                    nc.vector.tensor_copy(ob2[:rows, tq % OBN, :],
                                          ps[:rows, :])
                else:
                    nc.scalar.copy(ob2[:rows, tq % OBN, :], ps[:rows, :])
                if tq % OBN == OBN - 1 or tq == TQ - 1:
                    base = tq - tq % OBN
                    pending_stores.append((128 * base,
                                           128 * (tq % OBN) + rows, ob2))

            # 5-stage pipeline: S(j) | sm_a(j-1) | sm_b(j-2) | slack | bk(j-4)
            its = [(wg2, c) for wg2 in range(WG2) for c in range(CH)]
            NIT = len(its)
            stage_s, stage_p, stage_t = {}, {}, {}
            state = {"done": 0, "ready": 0}

            def op_some(nmax):
                while state["done"] < state["ready"] and nmax > 0:
                    outproj(state["done"])
                    state["done"] += 1
                    nmax -= 1

            ready_updates = []
            for j in range(NIT + 4):
                # outproj first: its PSUM is drained early in the body so the
                # ACT drain never gates this body's exp chain.  Tiles become
                # eligible two bodies after their supergroup's last BK so the
                # attnT drains are never chased.
                flush_stores()
                for (eb, rv) in list(ready_updates):
                    if j >= eb:
                        state["ready"] = max(state["ready"], rv)
                        ready_updates.remove((eb, rv))
                op_some(2 if j < NIT else 3)
                if j < NIT:
                    stage_s[j] = s_stage(*its[j])
                    stage_p[j] = sm_a(stage_s.pop(j), use_dve=(j >= NIT - 2))
                if j == 6:
                    preamble()
                if 0 <= j - 2 < NIT:
                    stage_t[j - 2] = sm_b(stage_p.pop(j - 2))
                if 0 <= j - 4 < NIT:
                    i = j - 4
                    bit = its[i]
                    bk_stage(bit[0], bit[1], stage_t.pop(i))
                    if bit[1] == CH - 1:
                        rv = TQ if bit[0] == WG2 - 1 else 8 * (bit[0] + 1)
                        ready_updates.append((j + 1, rv))
            state["ready"] = TQ
            op_some(TQ)
            flush_stores()


def build(T=T_FULL):
    nc = bacc.Bacc("TRN2", target_bir_lowering=False, debug=False,
                   num_devices=N_CORES)
    x_d = nc.dram_tensor("x", [T, D], F32, kind="ExternalInput")
    wqkv_d = nc.dram_tensor("w_qkv", [D, 3 * D], F32, kind="ExternalInput")
    wout_d = nc.dram_tensor("w_out", [D, D], F32, kind="ExternalInput")
    out_d = nc.dram_tensor("out", [T, D], F32, kind="ExternalOutput")
    with tile.TileContext(nc) as tc:
        _emit(nc, tc, x_d.ap(), wqkv_d.ap(), wout_d.ap(), out_d.ap(), T)
    nc.compile()
    return nc


_NC_CACHE = {}


def kernel(x, w_qkv, w_out):
    x = np.ascontiguousarray(np.asarray(x, dtype=np.float32))
    w_qkv = np.ascontiguousarray(np.asarray(w_qkv, dtype=np.float32))
    w_out = np.ascontiguousarray(np.asarray(w_out, dtype=np.float32))
    assert x.shape == (B, T_FULL, D)

    if "nc" not in _NC_CACHE:
        _NC_CACHE["nc"] = build(T_FULL)
    nc = _NC_CACHE["nc"]

    in_maps = [
        {"x": x[b], "w_qkv": w_qkv, "w_out": w_out} for b in range(N_CORES)
    ]
    last_err = None
    for _attempt in range(4):
        try:
            res = run_bass_kernel_spmd(nc, in_maps, core_ids=list(range(N_CORES)))
            break
        except Exception as e:  # transient NRT device errors
            last_err = e
            try:  # force a fresh PJRT client before retrying
                import jax
                jax.clear_caches()
                jax.extend.backend.clear_backends()
            except Exception:
                pass
            import time as _time
            _time.sleep(5)
    else:
        raise last_err
    return np.stack([res.results[b]["out"] for b in range(N_CORES)], axis=0)


# revision 76
# speedup vs baseline: 1.0208x; 1.0023x over previous
"""BBox window attention kernel for 8 TRN2 NeuronCores.

Sharding: data-parallel over batch B=8 -> one batch element per core.
Each core computes the full attention for its batch element; no collectives.

Per-core pipeline (all matmuls bf16 with f32 PSUM accumulation):
  1. Phase A streams x in 512-token blocks SHIFTED BY ONE TOKEN (tokens
     1..4096) so windows/v tiles align with block boundaries; token 0 runs
     through a tiny separate path.  Per block: DMA f32 -> DVE cast bf16 ->
     PE transposes (PSUM, drained on ACT) -> xT tile-major feature-major;
     then qkT = W_qk^T @ xT (feature-major q,k) and v = xT^T @ W_v
     (token-major).  Loads lead casts by ~3 blocks, casts lead their
     consumers by a section; dummy ident matmuls warm the PE p-state
     through the DMA startup.
  2. Global token, transposed path: s0T[t,h] via 8-col matmuls, exp ->
     P0T, denominators via ones-matmul (contraction over partitions), o0T
     via v-as-stationary 8-col matmuls, partials accumulated in SBUF.
     Normalization is deferred to the attnT scatter (ACT activation with a
     per-partition scale built by a selector matmul; the selector mask is
     generated with affine_select band conditions).
  3. Windows, 5-stage emission pipeline (S | softmax | P-xbar | slack |
     PV): S matmuls (2 PSUM banks split by head-half) -> exp (ACT,
     unnormalized, both halves into one P tile) -> DVE reduce+recip ->
     Pool broadcast-normalize -> ONE dma_start_transpose of P per
     iteration (SP queue) -> V^T @ P^T -> attnT (feature-major), drained
     on DVE.  Engine roles are kept homogeneous so the Tile scheduler's
     frozen per-engine orders cannot head-of-line-block the chain.
  4. out = attnT^T @ W_out interleaved 2 tiles/iteration as supergroups
     complete; PSUM drained on ACT into 4-tile batches, stores on the SP
     queue deferred one body so dispatches never block on data.
"""

import sys

for _p in ("/opt/trn_rl_repo",):
    if _p not in sys.path:
        sys.path.insert(0, _p)

import numpy as np

import concourse.bass as bass
import concourse.tile as tile
from concourse import bacc, mybir
from concourse.bass_utils import run_bass_kernel_spmd
from concourse.masks import make_identity

F32 = mybir.dt.float32
BF16 = mybir.dt.bfloat16
EXP = mybir.ActivationFunctionType.Exp
IDENT = mybir.ActivationFunctionType.Identity

B, T_FULL, D = 8, 4097, 512
H, WIN, d_head = 8, 64, 64
N_CORES = 8
CH = 4          # head-pair chunks (128 features each)
KC = 4          # contraction chunks of 128 over D
SCALE = float(d_head) ** -0.5
N_WARM = 22
N_WARM2 = 80    # PE p-state warmup matmuls (128-col) covering DMA startup


def _emit(nc, tc, x_d, wqkv_d, wout_d, out_d, T):
    TW = T - 1                  # window tokens (4096)
    NB = TW // 512              # x blocks of 512 tokens
    VT = TW // 128              # v tiles
    WG2 = (TW // WIN) // 16     # supergroups of 16 windows
    TQ = (T + 127) // 128       # output tiles
    assert TW % 512 == 0

    def pool(name, **kw):
        return tc.tile_pool(name=name, **kw)

    with pool("persist", bufs=1) as persist, \
         pool("stats", bufs=4) as stats:

        ident = persist.tile([128, 128], BF16)
        make_identity(nc, ident)

        wqkv_sb = persist.tile([128, KC, 3 * D], BF16)
        wout_sb = persist.tile([128, KC, D], BF16)
        qT = persist.tile([128, CH, T], BF16)
        kT = persist.tile([128, CH, T], BF16)
        v_sb = persist.tile([128, VT, D], BF16)
        v0_sb = persist.tile([1, D], BF16)
        q0all = persist.tile([128, CH, 8], BF16)
        P0T = persist.tile([128, VT, 8], BF16)
        p00 = persist.tile([1, 8], BF16)
        ones_sb = persist.tile([128, 1], BF16)
        o0acc = persist.tile([128, CH, 8], F32)
        s0r = persist.tile([8, 1], F32)

        nc.vector.memset(ones_sb[:, :], 1.0)
        nc.vector.memset(q0all[:, :, :], 0.0)

        # ---- phase A: weights, x load/cast/xbar-transpose, projections ----
        with pool("xstage", bufs=2) as xstage, \
             pool("xTp", bufs=1) as xTp, \
             pool("pA", bufs=8, space="PSUM") as pA:

            # p-state warmup: keep the PE continuously busy from t~1us until
            # the first projection matmuls are ready, so the dispatch-time
            # ramp model reaches full clock before real work arrives.
            warm_ps = pA.tile([128, 128], F32, tag="pa")
            for _ in range(N_WARM):
                nc.tensor.matmul(warm_ps[:, :], ident[:, :], ident[:, :],
                                 start=True, stop=True)

            # xT[p, tt, kc, tc] = x[1 + 128*tt + tc, 128*kc + p]
            xT = xTp.tile([128, NB * 4, KC, 128], BF16)
            s0acc = xstage.tile([8, 1], F32, tag="s0acc", bufs=1)
            nc.vector.memset(s0acc[:, :], 0.0)
            nc.vector.memset(o0acc[:, :, :], 0.0)

            def load_wqkv(hh):
                for kc in range(KC):
                    st = xstage.tile([128, 768], F32, tag="wst", bufs=3)
                    nc.sync.dma_start(
                        out=st[:, :],
                        in_=wqkv_d[128 * kc:128 * kc + 128,
                                   768 * hh:768 * hh + 768],
                    )
                    nc.vector.tensor_copy(
                        wqkv_sb[:, kc, 768 * hh:768 * hh + 768], st[:, :]
                    )

            def load_wout():
                for kc in range(KC):
                    st = xstage.tile([128, 512], F32, tag="wst", bufs=3)
                    nc.sync.dma_start(
                        out=st[:, :], in_=wout_d[128 * kc:128 * kc + 128, :]
                    )
                    nc.vector.tensor_copy(wout_sb[:, kc, :], st[:, :])

            xs_tiles = {}

            def load_block(b):
                xs = xstage.tile([128, 4, D], F32, tag="xs", name="xs", bufs=3)
                nc.sync.dma_start(
                    out=xs[:, :, :],
                    in_=x_d[1 + 512 * b:1 + 512 * b + 512, :].rearrange(
                        "(j p) e -> p j e", p=128),
                )
                xs_tiles[b] = xs

            xc_tiles = {}

            def cast(b):
                # cast on DVE (leads its consumers by a full section)
                xc = xstage.tile([128, 4, D], BF16, tag="xc", name="xc",
                                 bufs=3)
                nc.vector.tensor_copy(xc[:, :, :], xs_tiles.pop(b)[:, :, :])
                xc_tiles[b] = xc

            def tp_drain(b):
                # transpose on PE (self-paced), drain on ACT
                xc = xc_tiles.pop(b)
                for j2 in range(4):
                    tp = pA.tile([128, KC, 128], BF16, tag="pa", name="tp")
                    for kc in range(KC):
                        nc.tensor.transpose(
                            tp[:, kc, :],
                            xc[:, j2, 128 * kc:128 * kc + 128],
                            ident[:, :],
                        )
                    nc.scalar.copy(xT[:, 4 * b + j2, :, :], tp[:, :, :])

            x0_tiles = {}

            def x0_load():
                xs0 = xstage.tile([1, D], F32, tag="xs0", bufs=1)
                nc.sync.dma_start(out=xs0[:, :], in_=x_d[0:1, :])
                xc0 = xstage.tile([1, D], BF16, tag="xc0", bufs=1)
                nc.scalar.copy(xc0[:, :], xs0[:, :])
                x0_tiles["xc0"] = xc0

            def x0_path():
                xc0 = x0_tiles["xc0"]
                tp0 = pA.tile([128, KC, 2], BF16, tag="pa")
                for kc in range(KC):
                    nc.tensor.transpose(
                        tp0[:, kc, 0:1], xc0[:, 128 * kc:128 * kc + 128],
                        ident[0:1, 0:1],
                    )
                xT0 = xstage.tile([128, KC, 1], BF16, tag="xT0", bufs=1)
                nc.vector.tensor_copy(xT0[:, :, :], tp0[:, :, 0:1])
                qk0ps = pA.tile([128, 8], F32, tag="pa")
                for jb in range(8):
                    for kc in range(KC):
                        nc.tensor.matmul(
                            qk0ps[:, jb:jb + 1],
                            wqkv_sb[:, kc, 128 * jb:128 * jb + 128],
                            xT0[:, kc, :],
                            start=(kc == 0), stop=(kc == KC - 1),
                        )
                q0sb = xstage.tile([128, 8], BF16, tag="q0sb", bufs=1)
                nc.vector.tensor_copy(q0sb[:, :], qk0ps[:, :])
                for c in range(CH):
                    nc.vector.tensor_copy(kT[:, c, 0:1], q0sb[:, 4 + c:5 + c])
                for h in range(H):
                    rr = 64 * (h % 2)
                    nc.vector.tensor_copy(
                        q0all[rr:rr + 64, h // 2, h:h + 1],
                        q0sb[rr:rr + 64, h // 2:h // 2 + 1],
                    )
                v0ps = pA.tile([1, D], F32, tag="pa")
                for kc in range(KC):
                    nc.tensor.matmul(
                        v0ps[:, :], xT0[:, kc, :], wqkv_sb[:, kc, 2 * D:3 * D],
                        start=(kc == 0), stop=(kc == KC - 1),
                    )
                nc.vector.tensor_copy(v0_sb[:, :], v0ps[:, :])

            def qkproj(b, jbs):
                c0 = 1 + 512 * b
                for jb in jbs:
                    ps = pA.tile([128, 512], F32, tag="pa")
                    for kc in range(KC):
                        nc.tensor.matmul(
                            ps[:, :],
                            wqkv_sb[:, kc, 128 * jb:128 * jb + 128],
                            xT[:, 4 * b:4 * b + 4, kc, :],
                            start=(kc == 0), stop=(kc == KC - 1),
                        )
                    dst = (qT if jb < 4 else kT)[:, jb % 4, c0:c0 + 512]
                    if jb < 6:
                        nc.vector.tensor_copy(dst, ps[:, :])
                    else:
                        nc.scalar.copy(dst, ps[:, :])

            def vproj(b):
                for j2 in range(4):
                    vt = 4 * b + j2
                    ps = pA.tile([128, D], F32, tag="pa")
                    for kc in range(KC):
                        nc.tensor.matmul(
                            ps[:, :],
                            xT[:, vt, kc, :],
                            wqkv_sb[:, kc, 2 * D:3 * D],
                            start=(kc == 0), stop=(kc == KC - 1),
                        )
                    nc.vector.tensor_copy(v_sb[:, vt, :], ps[:, :])

            def s0t(b):
                # s0T[t, h] for tokens of block b; exp into P0T (unnormalized)
                ps = pA.tile([128, 4, 8], F32, tag="pa")
                for j2 in range(4):
                    vt = 4 * b + j2
                    t0 = 1 + 128 * vt
                    for c in range(CH):
                        nc.tensor.matmul(
                            ps[:, j2, :],
                            kT[:, c, t0:t0 + 128],
                            q0all[:, c, :],
                            start=(c == 0), stop=(c == CH - 1),
                        )
                nc.scalar.activation(
                    P0T[:, 4 * b:4 * b + 4, :].rearrange("p a b -> p (a b)"),
                    ps[:, :, :].rearrange("p a b -> p (a b)"),
                    EXP, bias=0.0, scale=SCALE,
                )

            def sums_o0(b):
                # denominators + o0T contributions for block b (emitted one
                # block late so v/P0T drains are long done); per-block psum
                # partials accumulated into SBUF so no PSUM bank is pinned
                s0p = pA.tile([8, 1], F32, tag="pa", name="s0p")
                o0p = pA.tile([128, CH, 8], F32, tag="pa", name="o0p")
                for j2 in range(4):
                    vt = 4 * b + j2
                    nc.tensor.matmul(
                        s0p[:, :], P0T[:, vt, :], ones_sb[:, :],
                        start=(j2 == 0), stop=(j2 == 3),
                    )
                    for fb in range(CH):
                        nc.tensor.matmul(
                            o0p[:, fb, :],
                            v_sb[:, vt, 128 * fb:128 * fb + 128],
                            P0T[:, vt, :],
                            start=(j2 == 0), stop=(j2 == 3),
                        )
                nc.vector.tensor_tensor(s0acc[:, :], s0acc[:, :], s0p[:, :],
                                        op=mybir.AluOpType.add)
                nc.vector.tensor_tensor(o0acc[:, :, :], o0acc[:, :, :],
                                        o0p[:, :, :],
                                        op=mybir.AluOpType.add)

            # emission order = scheduler priority; DMAs are emitted in true
            # readiness order (loads lead casts/xbars, which lead computes)
            load_block(0)
            cast(0)
            load_wqkv(0)
            for _ in range(N_WARM2):
                nc.tensor.matmul(warm_ps[:, :], ident[:, :], ident[:, :],
                                 start=True, stop=True)
            tp_drain(0)
            load_wqkv(1)
            x0_load()
            load_block(1)
            cast(1)
            load_wout()
            load_block(2)
            for b in range(NB):
                if b + 3 < NB:
                    load_block(b + 3)
                if b + 2 < NB:
                    cast(b + 2)
                qkproj(b, range(0, 4))
                if b + 1 < NB and b > 0:
                    tp_drain(b + 1)
                qkproj(b, range(4, 8))
                if b == 0:
                    tp_drain(1)
                vproj(b)
                if b == 1:
                    x0_path()
                if b > 0:
                    s0t(b - 1)
                if b > 1:
                    sums_o0(b - 2)
            s0t(NB - 1)
            sums_o0(NB - 2)
            sums_o0(NB - 1)

            # token-0 key column: s00 -> p00; close the accumulation groups
            s00ps = pA.tile([1, 8], F32, tag="pa")
            for c in range(CH):
                nc.tensor.matmul(
                    s00ps[:, :], kT[:, c, 0:1], q0all[:, c, :],
                    start=(c == 0), stop=(c == CH - 1),
                )
            nc.scalar.activation(p00[:, :], s00ps[:, :], EXP,
                                 bias=0.0, scale=SCALE)
            s0p0 = pA.tile([8, 1], F32, tag="pa", name="s0p0")
            o0p0 = pA.tile([128, CH, 8], F32, tag="pa", name="o0p0")
            nc.tensor.matmul(s0p0[:, :], p00[:, :], ones_sb[0:1, :],
                             start=True, stop=True)
            for fb in range(CH):
                nc.tensor.matmul(
                    o0p0[:, fb, :],
                    v0_sb[:, 128 * fb:128 * fb + 128],
                    p00[:, :],
                    start=True, stop=True,
                )
            nc.vector.tensor_tensor(s0acc[:, :], s0acc[:, :], s0p0[:, :],
                                    op=mybir.AluOpType.add)
            nc.vector.tensor_tensor(o0acc[:, :, :], o0acc[:, :, :],
                                    o0p0[:, :, :], op=mybir.AluOpType.add)
            nc.vector.reciprocal(s0r[:, :], s0acc[:, :])

        # ---- windows + output projection ----
        with pool("attnp", bufs=1) as attnp, \
             pool("pp", bufs=4) as ppool, \
             pool("ptp", bufs=4) as ptp, \
             pool("wstats", bufs=4) as wstats, \
             pool("osb", bufs=4) as posb, \
             pool("prow0", bufs=5, space="PSUM") as prow0, \
             pool("prow64", bufs=3, space="PSUM") as prow64:

            attnT = attnp.tile([128, CH, T], BF16)
            selT = attnp.tile([8, CH, 128], F32)
            rep_sb = attnp.tile([128, CH], F32)

            def preamble():
                # scatter o0 into attnT column 0, normalized by 1/s0sum via
                # a per-partition scale vector built by a selector matmul
                # selT[h, c, p] = 1 iff h == 2c + (p >= 64), built with two
                # affine band selects per chunk (partition-aligned accesses)
                nc.gpsimd.memset(selT[:, :, :], 1.0)
                for c in range(CH):
                    nc.gpsimd.affine_select(
                        out=selT[:, c, :], in_=selT[:, c, :],
                        compare_op=mybir.AluOpType.is_ge, fill=0.0,
                        base=63 - 128 * c,
                        pattern=[[-1, 128]], channel_multiplier=64,
                    )
                    nc.gpsimd.affine_select(
                        out=selT[:, c, :], in_=selT[:, c, :],
                        compare_op=mybir.AluOpType.is_ge, fill=0.0,
                        base=128 * c,
                        pattern=[[1, 128]], channel_multiplier=-64,
                    )
                rep_ps = prow0.tile([128, CH], F32, tag="op", bufs=2)
                for c in range(CH):
                    nc.tensor.matmul(rep_ps[:, c:c + 1], selT[:, c, :],
                                     s0r[:, :], start=True, stop=True)
                nc.vector.tensor_copy(rep_sb[:, :], rep_ps[:, :])
                for c in range(CH):
                    nc.scalar.activation(
                        attnT[0:64, c, 0:1], o0acc[0:64, c, 2 * c:2 * c + 1],
                        IDENT, bias=0.0, scale=rep_sb[0:64, c:c + 1])
                    nc.scalar.activation(
                        attnT[64:128, c, 0:1],
                        o0acc[64:128, c, 2 * c + 1:2 * c + 2],
                        IDENT, bias=0.0, scale=rep_sb[64:128, c:c + 1])

            # Window wj (0..15 in a supergroup) maps to (u, b1, s2) =
            # (wj&1, (wj>>1)&1, wj>>2).  Layouts (hardware-validated):
            #   S tile (per head-half r):  [64*b1 + q, slot=2*s2+u, k]
            #   PT (transposed P):         [64*u + k, slab=4*r+s2, 64*b1 + q]
            #   O tile (per parity u):     [64*r + e, slot=2*s2+b1, q]
            def s_stage(wg2, c):
                banks = []
                for r in range(2):
                    sp = (prow0 if r == 0 else prow64).tile(
                        [128, 8, WIN], F32, bufs=2,
                        tag=("S0" if r == 0 else "S1"))
                    for wj in range(16):
                        u, b1, s2 = wj & 1, (wj >> 1) & 1, wj >> 2
                        col0 = 1 + WIN * (16 * wg2 + wj)
                        nc.tensor.matmul(
                            sp[64 * b1:64 * b1 + 64, 2 * s2 + u, :],
                            qT[64 * r:64 * r + 64, c, col0:col0 + WIN],
                            kT[64 * r:64 * r + 64, c, col0:col0 + WIN],
                            start=True, stop=True,
                        )
                    banks.append(sp)
                return banks

            def sm_a(banks, use_dve=False):
                # exp (unnormalized) + sums + recip + Pool normalize.  Both
                # head-half banks land in one P tile so sm_b is a single xbar.
                pb = ppool.tile([128, 2, 8, WIN], BF16, tag="P")
                sums = wstats.tile([128, 2, 8, 1], F32, tag="sums")
                for r in range(2):
                    nc.scalar.activation(
                        pb[:, r, :, :].rearrange("p a b -> p (a b)"),
                        banks[r][:, :, :].rearrange("p a b -> p (a b)"),
                        EXP, bias=0.0, scale=SCALE,
                    )
                    nc.vector.reduce_sum(
                        sums[:, r, :, :], pb[:, r, :, :],
                        axis=mybir.AxisListType.X,
                        op=mybir.AluOpType.add,
                    )
                rs = wstats.tile([128, 2, 8, 1], F32, tag="rs")
                nc.vector.reciprocal(rs[:, :, :, :], sums[:, :, :, :])
                eng = nc.vector if use_dve else nc.gpsimd
                eng.tensor_tensor(
                    pb[:, :, :, :], pb[:, :, :, :],
                    rs[:, :, :, :].broadcast_to([128, 2, 8, WIN]),
                    op=mybir.AluOpType.mult,
                )
                return pb

            def sm_b(pb):
                PT_sb = ptp.tile([128, 8, 128], BF16, tag="PT")
                nc.sync.dma_start_transpose(
                    out=PT_sb[:, :, :], in_=pb[:, :, :, :]
                )
                return PT_sb

            def bk_stage(wg2, c, PT_sb):
                cb = 1 + 1024 * wg2
                av = attnT[:, c, cb:cb + 1024].rearrange(
                    "p (a b u q) -> p a b u q", a=4, b=2, u=2)
                for u in range(2):
                    op = (prow0 if u == 0 else prow64).tile(
                        [128, 8, WIN], F32, bufs=1,
                        tag=("O0" if u == 0 else "O1"))
                    for b1 in range(2):
                        for s2 in range(4):
                            wp = 8 * wg2 + 2 * s2 + b1
                            for r in range(2):
                                h = 2 * c + r
                                nc.tensor.matmul(
                                    op[64 * r:64 * r + 64, 2 * s2 + b1, :],
                                    v_sb[64 * u:64 * u + 64, wp,
                                         64 * h:64 * h + 64],
                                    PT_sb[64 * u:64 * u + 64, 4 * r + s2,
                                          64 * b1:64 * b1 + 64],
                                    start=True, stop=True,
                                )
                    nc.vector.tensor_copy(
                        av[:, :, :, u, :],
                        op[:, :, :].rearrange("p (a b) q -> p a b q", a=4),
                    )

            ob_state = {}
            OBN = 8
            pending_stores = []

            def flush_stores():
                # store dispatches deferred a body so the SP queue never
                # blocks on drain data (SP also carries the PT xbars)
                for rr, nrows, ob in pending_stores:
                    full, tail = nrows // 128, nrows % 128
                    if full:
                        nc.sync.dma_start(
                            out=out_d[rr:rr + 128 * full, :].rearrange(
                                "(j p) e -> p j e", p=128),
                            in_=ob[:, 0:full, :],
                        )
                    if tail:
                        nc.sync.dma_start(
                            out=out_d[rr + 128 * full:rr + 128 * full + tail,
                                      :],
                            in_=ob[:tail, full, :])
                del pending_stores[:]

            def outproj(tq):
                r0 = 128 * tq
                rows = min(128, T - r0)
                ps = prow0.tile([128, D], F32, tag="op", bufs=2)
                for c in range(CH):
                    nc.tensor.matmul(
                        ps[:rows, :],
                        attnT[:, c, r0:r0 + rows],
                        wout_sb[:, c, :],
                        start=(c == 0), stop=(c == CH - 1),
                    )
                # drains on ACT (latency-tolerant); DVE keeps the softmax path
                if tq % OBN == 0:
                    ob_state["t"] = posb.tile([128, OBN, D], F32, tag="ob",
                                              name="ob4", bufs=2)
                ob2 = ob_state["t"]
                if tq >= 24 and tq % 2 == 1:
                    nc.vector.tensor_copy(ob2[:rows, tq % OBN, :],
                                          ps[:rows, :])
                else:
                    nc.scalar.copy(ob2[:rows, tq % OBN, :], ps[:rows, :])
                if tq % OBN == OBN - 1 or tq == TQ - 1:
                    base = tq - tq % OBN
                    pending_stores.append((128 * base,
                                           128 * (tq % OBN) + rows, ob2))

            # 5-stage pipeline: S(j) | sm_a(j-1) | sm_b(j-2) | slack | bk(j-4)
            its = [(wg2, c) for wg2 in range(WG2) for c in range(CH)]
            NIT = len(its)
            stage_s, stage_p, stage_t = {}, {}, {}
            state = {"done": 0, "ready": 0}

            def op_some(nmax):
                while state["done"] < state["ready"] and nmax > 0:
                    outproj(state["done"])
                    state["done"] += 1
                    nmax -= 1

            ready_updates = []
            for j in range(NIT + 4):
                # outproj first: its PSUM is drained early in the body so the
                # ACT drain never gates this body's exp chain.  Tiles become
                # eligible two bodies after their supergroup's last BK so the
                # attnT drains are never chased.
                flush_stores()
                for (eb, rv) in list(ready_updates):
                    if j >= eb:
                        state["ready"] = max(state["ready"], rv)
                        ready_updates.remove((eb, rv))
                op_some(2 if j < NIT else 3)
                if j < NIT:
                    stage_s[j] = s_stage(*its[j])
                    stage_p[j] = sm_a(stage_s.pop(j), use_dve=(j >= NIT - 2))
                if j == 3:
                    preamble()
                if 0 <= j - 2 < NIT:
                    stage_t[j - 2] = sm_b(stage_p.pop(j - 2))
                if 0 <= j - 4 < NIT:
                    i = j - 4
                    bit = its[i]
                    bk_stage(bit[0], bit[1], stage_t.pop(i))
                    if bit[1] == CH - 1:
                        rv = TQ if bit[0] == WG2 - 1 else 8 * (bit[0] + 1)
                        ready_updates.append((j + 1, rv))
            state["ready"] = TQ
            op_some(TQ)
            flush_stores()


def build(T=T_FULL):
    nc = bacc.Bacc("TRN2", target_bir_lowering=False, debug=False,
                   num_devices=N_CORES)
    x_d = nc.dram_tensor("x", [T, D], F32, kind="ExternalInput")
    wqkv_d = nc.dram_tensor("w_qkv", [D, 3 * D], F32, kind="ExternalInput")
    wout_d = nc.dram_tensor("w_out", [D, D], F32, kind="ExternalInput")
    out_d = nc.dram_tensor("out", [T, D], F32, kind="ExternalOutput")
    with tile.TileContext(nc) as tc:
        _emit(nc, tc, x_d.ap(), wqkv_d.ap(), wout_d.ap(), out_d.ap(), T)
    nc.compile()
    return nc


_NC_CACHE = {}


def kernel(x, w_qkv, w_out):
    x = np.ascontiguousarray(np.asarray(x, dtype=np.float32))
    w_qkv = np.ascontiguousarray(np.asarray(w_qkv, dtype=np.float32))
    w_out = np.ascontiguousarray(np.asarray(w_out, dtype=np.float32))
    assert x.shape == (B, T_FULL, D)

    if "nc" not in _NC_CACHE:
        _NC_CACHE["nc"] = build(T_FULL)
    nc = _NC_CACHE["nc"]

    in_maps = [
        {"x": x[b], "w_qkv": w_qkv, "w_out": w_out} for b in range(N_CORES)
    ]
    last_err = None
    for _attempt in range(4):
        try:
            res = run_bass_kernel_spmd(nc, in_maps, core_ids=list(range(N_CORES)))
            break
        except Exception as e:  # transient NRT device errors
            last_err = e
            try:  # force a fresh PJRT client before retrying
                import jax
                jax.clear_caches()
                jax.extend.backend.clear_backends()
            except Exception:
                pass
            import time as _time
            _time.sleep(5)
    else:
        raise last_err
    return np.stack([res.results[b]["out"] for b in range(N_CORES)], axis=0)


# revision 77
# speedup vs baseline: 1.0299x; 1.0089x over previous
"""BBox window attention kernel for 8 TRN2 NeuronCores.

Sharding: data-parallel over batch B=8 -> one batch element per core.
Each core computes the full attention for its batch element; no collectives.

Per-core pipeline (all matmuls bf16 with f32 PSUM accumulation):
  1. Phase A streams x in 512-token blocks SHIFTED BY ONE TOKEN (tokens
     1..4096) so windows/v tiles align with block boundaries; token 0 runs
     through a tiny separate path.  Per block: DMA f32 -> DVE cast bf16 ->
     PE transposes (PSUM, drained on ACT) -> xT tile-major feature-major;
     then qkT = W_qk^T @ xT (feature-major q,k) and v = xT^T @ W_v
     (token-major).  Loads lead casts by ~3 blocks, casts lead their
     consumers by a section; dummy ident matmuls warm the PE p-state
     through the DMA startup.
  2. Global token, transposed path: s0T[t,h] via 8-col matmuls, exp ->
     P0T, denominators via ones-matmul (contraction over partitions), o0T
     via v-as-stationary 8-col matmuls, partials accumulated in SBUF.
     Normalization is deferred to the attnT scatter (ACT activation with a
     per-partition scale built by a selector matmul; the selector mask is
     generated with affine_select band conditions).
  3. Windows, 5-stage emission pipeline (S | softmax | P-xbar | slack |
     PV): S matmuls (2 PSUM banks split by head-half) -> exp (ACT,
     unnormalized, both halves into one P tile) -> DVE reduce+recip ->
     Pool broadcast-normalize -> ONE dma_start_transpose of P per
     iteration (SP queue) -> V^T @ P^T -> attnT (feature-major), drained
     on DVE.  Engine roles are kept homogeneous so the Tile scheduler's
     frozen per-engine orders cannot head-of-line-block the chain.
  4. out = attnT^T @ W_out interleaved 2 tiles/iteration as supergroups
     complete; PSUM drained on ACT into 4-tile batches, stores on the SP
     queue deferred one body so dispatches never block on data.
"""

import sys

for _p in ("/opt/trn_rl_repo",):
    if _p not in sys.path:
        sys.path.insert(0, _p)

import numpy as np

import concourse.bass as bass
import concourse.tile as tile
from concourse import bacc, mybir
from concourse.bass_utils import run_bass_kernel_spmd
from concourse.masks import make_identity

F32 = mybir.dt.float32
BF16 = mybir.dt.bfloat16
EXP = mybir.ActivationFunctionType.Exp
IDENT = mybir.ActivationFunctionType.Identity

B, T_FULL, D = 8, 4097, 512
H, WIN, d_head = 8, 64, 64
N_CORES = 8
CH = 4          # head-pair chunks (128 features each)
KC = 4          # contraction chunks of 128 over D
SCALE = float(d_head) ** -0.5
N_WARM = 22
N_WARM2 = 80    # PE p-state warmup matmuls (128-col) covering DMA startup


def _emit(nc, tc, x_d, wqkv_d, wout_d, out_d, T):
    TW = T - 1                  # window tokens (4096)
    NB = TW // 512              # x blocks of 512 tokens
    VT = TW // 128              # v tiles
    WG2 = (TW // WIN) // 16     # supergroups of 16 windows
    TQ = (T + 127) // 128       # output tiles
    assert TW % 512 == 0

    def pool(name, **kw):
        return tc.tile_pool(name=name, **kw)

    with pool("persist", bufs=1) as persist, \
         pool("stats", bufs=4) as stats:

        ident = persist.tile([128, 128], BF16)
        make_identity(nc, ident)

        wqkv_sb = persist.tile([128, KC, 3 * D], BF16)
        wout_sb = persist.tile([128, KC, D], BF16)
        qT = persist.tile([128, CH, T], BF16)
        kT = persist.tile([128, CH, T], BF16)
        v_sb = persist.tile([128, VT, D], BF16)
        v0_sb = persist.tile([1, D], BF16)
        q0all = persist.tile([128, CH, 8], BF16)
        P0T = persist.tile([128, VT, 8], BF16)
        p00 = persist.tile([1, 8], BF16)
        ones_sb = persist.tile([128, 1], BF16)
        o0acc = persist.tile([128, CH, 8], F32)
        s0r = persist.tile([8, 1], F32)

        nc.vector.memset(ones_sb[:, :], 1.0)
        nc.vector.memset(q0all[:, :, :], 0.0)

        # ---- phase A: weights, x load/cast/xbar-transpose, projections ----
        with pool("xstage", bufs=2) as xstage, \
             pool("xTp", bufs=1) as xTp, \
             pool("pA", bufs=8, space="PSUM") as pA:

            # p-state warmup: keep the PE continuously busy from t~1us until
            # the first projection matmuls are ready, so the dispatch-time
            # ramp model reaches full clock before real work arrives.
            warm_ps = pA.tile([128, 128], F32, tag="pa")
            for _ in range(N_WARM):
                nc.tensor.matmul(warm_ps[:, :], ident[:, :], ident[:, :],
                                 start=True, stop=True)

            # xT[p, tt, kc, tc] = x[1 + 128*tt + tc, 128*kc + p]
            xT = xTp.tile([128, NB * 4, KC, 128], BF16)
            s0acc = xstage.tile([8, 1], F32, tag="s0acc", bufs=1)
            nc.vector.memset(s0acc[:, :], 0.0)
            nc.vector.memset(o0acc[:, :, :], 0.0)

            def load_wqkv(hh):
                for kc in range(KC):
                    st = xstage.tile([128, 768], F32, tag="wst", bufs=3)
                    nc.sync.dma_start(
                        out=st[:, :],
                        in_=wqkv_d[128 * kc:128 * kc + 128,
                                   768 * hh:768 * hh + 768],
                    )
                    nc.vector.tensor_copy(
                        wqkv_sb[:, kc, 768 * hh:768 * hh + 768], st[:, :]
                    )

            def load_wout():
                for kc in range(KC):
                    st = xstage.tile([128, 512], F32, tag="wst", bufs=3)
                    nc.sync.dma_start(
                        out=st[:, :], in_=wout_d[128 * kc:128 * kc + 128, :]
                    )
                    nc.vector.tensor_copy(wout_sb[:, kc, :], st[:, :])

            xs_tiles = {}

            def load_block(b):
                xs = xstage.tile([128, 4, D], F32, tag="xs", name="xs", bufs=3)
                nc.sync.dma_start(
                    out=xs[:, :, :],
                    in_=x_d[1 + 512 * b:1 + 512 * b + 512, :].rearrange(
                        "(j p) e -> p j e", p=128),
                )
                xs_tiles[b] = xs

            xc_tiles = {}

            def cast(b):
                # cast on DVE (leads its consumers by a full section)
                xc = xstage.tile([128, 4, D], BF16, tag="xc", name="xc",
                                 bufs=3)
                nc.vector.tensor_copy(xc[:, :, :], xs_tiles.pop(b)[:, :, :])
                xc_tiles[b] = xc

            def tp_drain(b):
                # transpose on PE (self-paced), drain on ACT
                xc = xc_tiles.pop(b)
                for j2 in range(4):
                    tp = pA.tile([128, KC, 128], BF16, tag="pa", name="tp")
                    for kc in range(KC):
                        nc.tensor.transpose(
                            tp[:, kc, :],
                            xc[:, j2, 128 * kc:128 * kc + 128],
                            ident[:, :],
                        )
                    nc.scalar.copy(xT[:, 4 * b + j2, :, :], tp[:, :, :])

            x0_tiles = {}

            def x0_load():
                xs0 = xstage.tile([1, D], F32, tag="xs0", bufs=1)
                nc.sync.dma_start(out=xs0[:, :], in_=x_d[0:1, :])
                xc0 = xstage.tile([1, D], BF16, tag="xc0", bufs=1)
                nc.scalar.copy(xc0[:, :], xs0[:, :])
                x0_tiles["xc0"] = xc0

            def x0_path():
                xc0 = x0_tiles["xc0"]
                tp0 = pA.tile([128, KC, 2], BF16, tag="pa")
                for kc in range(KC):
                    nc.tensor.transpose(
                        tp0[:, kc, 0:1], xc0[:, 128 * kc:128 * kc + 128],
                        ident[0:1, 0:1],
                    )
                xT0 = xstage.tile([128, KC, 1], BF16, tag="xT0", bufs=1)
                nc.vector.tensor_copy(xT0[:, :, :], tp0[:, :, 0:1])
                qk0ps = pA.tile([128, 8], F32, tag="pa")
                for jb in range(8):
                    for kc in range(KC):
                        nc.tensor.matmul(
                            qk0ps[:, jb:jb + 1],
                            wqkv_sb[:, kc, 128 * jb:128 * jb + 128],
                            xT0[:, kc, :],
                            start=(kc == 0), stop=(kc == KC - 1),
                        )
                q0sb = xstage.tile([128, 8], BF16, tag="q0sb", bufs=1)
                nc.vector.tensor_copy(q0sb[:, :], qk0ps[:, :])
                for c in range(CH):
                    nc.vector.tensor_copy(kT[:, c, 0:1], q0sb[:, 4 + c:5 + c])
                for h in range(H):
                    rr = 64 * (h % 2)
                    nc.vector.tensor_copy(
                        q0all[rr:rr + 64, h // 2, h:h + 1],
                        q0sb[rr:rr + 64, h // 2:h // 2 + 1],
                    )
                v0ps = pA.tile([1, D], F32, tag="pa")
                for kc in range(KC):
                    nc.tensor.matmul(
                        v0ps[:, :], xT0[:, kc, :], wqkv_sb[:, kc, 2 * D:3 * D],
                        start=(kc == 0), stop=(kc == KC - 1),
                    )
                nc.vector.tensor_copy(v0_sb[:, :], v0ps[:, :])

            def qkproj(b, jbs):
                c0 = 1 + 512 * b
                for jb in jbs:
                    ps = pA.tile([128, 512], F32, tag="pa")
                    for kc in range(KC):
                        nc.tensor.matmul(
                            ps[:, :],
                            wqkv_sb[:, kc, 128 * jb:128 * jb + 128],
                            xT[:, 4 * b:4 * b + 4, kc, :],
                            start=(kc == 0), stop=(kc == KC - 1),
                        )
                    dst = (qT if jb < 4 else kT)[:, jb % 4, c0:c0 + 512]
                    if jb < 6:
                        nc.vector.tensor_copy(dst, ps[:, :])
                    else:
                        nc.scalar.copy(dst, ps[:, :])

            def vproj(b):
                for j2 in range(4):
                    vt = 4 * b + j2
                    ps = pA.tile([128, D], F32, tag="pa")
                    for kc in range(KC):
                        nc.tensor.matmul(
                            ps[:, :],
                            xT[:, vt, kc, :],
                            wqkv_sb[:, kc, 2 * D:3 * D],
                            start=(kc == 0), stop=(kc == KC - 1),
                        )
                    nc.vector.tensor_copy(v_sb[:, vt, :], ps[:, :])

            def s0t(b):
                # s0T[t, h] for tokens of block b; exp into P0T (unnormalized)
                ps = pA.tile([128, 4, 8], F32, tag="pa")
                for j2 in range(4):
                    vt = 4 * b + j2
                    t0 = 1 + 128 * vt
                    for c in range(CH):
                        nc.tensor.matmul(
                            ps[:, j2, :],
                            kT[:, c, t0:t0 + 128],
                            q0all[:, c, :],
                            start=(c == 0), stop=(c == CH - 1),
                        )
                nc.scalar.activation(
                    P0T[:, 4 * b:4 * b + 4, :].rearrange("p a b -> p (a b)"),
                    ps[:, :, :].rearrange("p a b -> p (a b)"),
                    EXP, bias=0.0, scale=SCALE,
                )

            def sums_o0(b):
                # denominators + o0T contributions for block b (emitted one
                # block late so v/P0T drains are long done); per-block psum
                # partials accumulated into SBUF so no PSUM bank is pinned
                s0p = pA.tile([8, 1], F32, tag="pa", name="s0p")
                o0p = pA.tile([128, CH, 8], F32, tag="pa", name="o0p")
                for j2 in range(4):
                    vt = 4 * b + j2
                    nc.tensor.matmul(
                        s0p[:, :], P0T[:, vt, :], ones_sb[:, :],
                        start=(j2 == 0), stop=(j2 == 3),
                    )
                    for fb in range(CH):
                        nc.tensor.matmul(
                            o0p[:, fb, :],
                            v_sb[:, vt, 128 * fb:128 * fb + 128],
                            P0T[:, vt, :],
                            start=(j2 == 0), stop=(j2 == 3),
                        )
                nc.vector.tensor_tensor(s0acc[:, :], s0acc[:, :], s0p[:, :],
                                        op=mybir.AluOpType.add)
                nc.vector.tensor_tensor(o0acc[:, :, :], o0acc[:, :, :],
                                        o0p[:, :, :],
                                        op=mybir.AluOpType.add)

            # emission order = scheduler priority; DMAs are emitted in true
            # readiness order (loads lead casts/xbars, which lead computes)
            load_block(0)
            cast(0)
            load_wqkv(0)
            for _ in range(N_WARM2):
                nc.tensor.matmul(warm_ps[:, :], ident[:, :], ident[:, :],
                                 start=True, stop=True)
            tp_drain(0)
            load_wqkv(1)
            x0_load()
            load_block(1)
            cast(1)
            load_wout()
            load_block(2)
            for b in range(NB):
                if b + 3 < NB:
                    load_block(b + 3)
                if b + 2 < NB:
                    cast(b + 2)
                qkproj(b, range(0, 4))
                if b + 1 < NB and b > 0:
                    tp_drain(b + 1)
                qkproj(b, range(4, 8))
                if b == 0:
                    tp_drain(1)
                vproj(b)
                if b == 1:
                    x0_path()
                if b > 0:
                    s0t(b - 1)
                if b > 1:
                    sums_o0(b - 2)
            s0t(NB - 1)
            sums_o0(NB - 2)
            sums_o0(NB - 1)

            # token-0 key column: s00 -> p00; close the accumulation groups
            s00ps = pA.tile([1, 8], F32, tag="pa")
            for c in range(CH):
                nc.tensor.matmul(
                    s00ps[:, :], kT[:, c, 0:1], q0all[:, c, :],
                    start=(c == 0), stop=(c == CH - 1),
                )
            nc.scalar.activation(p00[:, :], s00ps[:, :], EXP,
                                 bias=0.0, scale=SCALE)
            s0p0 = pA.tile([8, 1], F32, tag="pa", name="s0p0")
            o0p0 = pA.tile([128, CH, 8], F32, tag="pa", name="o0p0")
            nc.tensor.matmul(s0p0[:, :], p00[:, :], ones_sb[0:1, :],
                             start=True, stop=True)
            for fb in range(CH):
                nc.tensor.matmul(
                    o0p0[:, fb, :],
                    v0_sb[:, 128 * fb:128 * fb + 128],
                    p00[:, :],
                    start=True, stop=True,
                )
            nc.vector.tensor_tensor(s0acc[:, :], s0acc[:, :], s0p0[:, :],
                                    op=mybir.AluOpType.add)
            nc.vector.tensor_tensor(o0acc[:, :, :], o0acc[:, :, :],
                                    o0p0[:, :, :], op=mybir.AluOpType.add)
            nc.vector.reciprocal(s0r[:, :], s0acc[:, :])

        # ---- windows + output projection ----
        with pool("attnp", bufs=1) as attnp, \
             pool("pp", bufs=4) as ppool, \
             pool("ptp", bufs=4) as ptp, \
             pool("wstats", bufs=4) as wstats, \
             pool("osb", bufs=4) as posb, \
             pool("prow0", bufs=5, space="PSUM") as prow0, \
             pool("prow64", bufs=3, space="PSUM") as prow64:

            attnT = attnp.tile([128, CH, T], BF16)
            selT = attnp.tile([8, CH, 128], F32)
            rep_sb = attnp.tile([128, CH], F32)

            def preamble():
                # scatter o0 into attnT column 0, normalized by 1/s0sum via
                # a per-partition scale vector built by a selector matmul
                # selT[h, c, p] = 1 iff h == 2c + (p >= 64), built with two
                # affine band selects per chunk (partition-aligned accesses)
                nc.gpsimd.memset(selT[:, :, :], 1.0)
                for c in range(CH):
                    nc.gpsimd.affine_select(
                        out=selT[:, c, :], in_=selT[:, c, :],
                        compare_op=mybir.AluOpType.is_ge, fill=0.0,
                        base=63 - 128 * c,
                        pattern=[[-1, 128]], channel_multiplier=64,
                    )
                    nc.gpsimd.affine_select(
                        out=selT[:, c, :], in_=selT[:, c, :],
                        compare_op=mybir.AluOpType.is_ge, fill=0.0,
                        base=128 * c,
                        pattern=[[1, 128]], channel_multiplier=-64,
                    )
                rep_ps = prow0.tile([128, CH], F32, tag="op", bufs=2)
                for c in range(CH):
                    nc.tensor.matmul(rep_ps[:, c:c + 1], selT[:, c, :],
                                     s0r[:, :], start=True, stop=True)
                nc.vector.tensor_copy(rep_sb[:, :], rep_ps[:, :])
                for c in range(CH):
                    nc.scalar.activation(
                        attnT[0:64, c, 0:1], o0acc[0:64, c, 2 * c:2 * c + 1],
                        IDENT, bias=0.0, scale=rep_sb[0:64, c:c + 1])
                    nc.scalar.activation(
                        attnT[64:128, c, 0:1],
                        o0acc[64:128, c, 2 * c + 1:2 * c + 2],
                        IDENT, bias=0.0, scale=rep_sb[64:128, c:c + 1])

            # Window wj (0..15 in a supergroup) maps to (u, b1, s2) =
            # (wj&1, (wj>>1)&1, wj>>2).  Layouts (hardware-validated):
            #   S tile (per head-half r):  [64*b1 + q, slot=2*s2+u, k]
            #   PT (transposed P):         [64*u + k, slab=4*r+s2, 64*b1 + q]
            #   O tile (per parity u):     [64*r + e, slot=2*s2+b1, q]
            def s_stage(wg2, c):
                banks = []
                for r in range(2):
                    sp = (prow0 if r == 0 else prow64).tile(
                        [128, 8, WIN], F32, bufs=2,
                        tag=("S0" if r == 0 else "S1"))
                    for wj in range(16):
                        u, b1, s2 = wj & 1, (wj >> 1) & 1, wj >> 2
                        col0 = 1 + WIN * (16 * wg2 + wj)
                        nc.tensor.matmul(
                            sp[64 * b1:64 * b1 + 64, 2 * s2 + u, :],
                            qT[64 * r:64 * r + 64, c, col0:col0 + WIN],
                            kT[64 * r:64 * r + 64, c, col0:col0 + WIN],
                            start=True, stop=True,
                        )
                    banks.append(sp)
                return banks

            def sm_a(banks, use_dve=False):
                # exp (unnormalized) + sums + recip + Pool normalize.  Both
                # head-half banks land in one P tile so sm_b is a single xbar.
                pb = ppool.tile([128, 2, 8, WIN], BF16, tag="P")
                sums = wstats.tile([128, 2, 8, 1], F32, tag="sums")
                for r in range(2):
                    nc.scalar.activation(
                        pb[:, r, :, :].rearrange("p a b -> p (a b)"),
                        banks[r][:, :, :].rearrange("p a b -> p (a b)"),
                        EXP, bias=0.0, scale=SCALE,
                    )
                    nc.vector.reduce_sum(
                        sums[:, r, :, :], pb[:, r, :, :],
                        axis=mybir.AxisListType.X,
                        op=mybir.AluOpType.add,
                    )
                rs = wstats.tile([128, 2, 8, 1], F32, tag="rs")
                nc.vector.reciprocal(rs[:, :, :, :], sums[:, :, :, :])
                eng = nc.vector if use_dve else nc.gpsimd
                eng.tensor_tensor(
                    pb[:, :, :, :], pb[:, :, :, :],
                    rs[:, :, :, :].broadcast_to([128, 2, 8, WIN]),
                    op=mybir.AluOpType.mult,
                )
                return pb

            def sm_b(pb):
                PT_sb = ptp.tile([128, 8, 128], BF16, tag="PT")
                nc.sync.dma_start_transpose(
                    out=PT_sb[:, :, :], in_=pb[:, :, :, :]
                )
                return PT_sb

            def bk_stage(wg2, c, PT_sb):
                cb = 1 + 1024 * wg2
                av = attnT[:, c, cb:cb + 1024].rearrange(
                    "p (a b u q) -> p a b u q", a=4, b=2, u=2)
                for u in range(2):
                    op = (prow0 if u == 0 else prow64).tile(
                        [128, 8, WIN], F32, bufs=1,
                        tag=("O0" if u == 0 else "O1"))
                    for b1 in range(2):
                        for s2 in range(4):
                            wp = 8 * wg2 + 2 * s2 + b1
                            for r in range(2):
                                h = 2 * c + r
                                nc.tensor.matmul(
                                    op[64 * r:64 * r + 64, 2 * s2 + b1, :],
                                    v_sb[64 * u:64 * u + 64, wp,
                                         64 * h:64 * h + 64],
                                    PT_sb[64 * u:64 * u + 64, 4 * r + s2,
                                          64 * b1:64 * b1 + 64],
                                    start=True, stop=True,
                                )
                    nc.vector.tensor_copy(
                        av[:, :, :, u, :],
                        op[:, :, :].rearrange("p (a b) q -> p a b q", a=4),
                    )

            ob_state = {}
            OBN = 8
            pending_stores = []

            def flush_stores():
                # store dispatches deferred a body so the SP queue never
                # blocks on drain data (SP also carries the PT xbars)
                for rr, nrows, ob in pending_stores:
                    full, tail = nrows // 128, nrows % 128
                    if full:
                        nc.sync.dma_start(
                            out=out_d[rr:rr + 128 * full, :].rearrange(
                                "(j p) e -> p j e", p=128),
                            in_=ob[:, 0:full, :],
                        )
                    if tail:
                        nc.sync.dma_start(
                            out=out_d[rr + 128 * full:rr + 128 * full + tail,
                                      :],
                            in_=ob[:tail, full, :])
                del pending_stores[:]

            def outproj(tq):
                r0 = 128 * tq
                rows = min(128, T - r0)
                ps = prow0.tile([128, D], F32, tag="op", bufs=2)
                for c in range(CH):
                    nc.tensor.matmul(
                        ps[:rows, :],
                        attnT[:, c, r0:r0 + rows],
                        wout_sb[:, c, :],
                        start=(c == 0), stop=(c == CH - 1),
                    )
                # drains on ACT (latency-tolerant); DVE keeps the softmax
                # path.  8-wide store batches early, 4-wide for the tail so
                # the final stores overlap the last outprojs.
                bases = [0, 8, 16, 24, 28, 32]
                base = max(x for x in bases if x <= tq)
                if tq == base:
                    ob_state["t"] = posb.tile([128, OBN, D], F32, tag="ob",
                                              name="ob4", bufs=2)
                    ob_state["base"] = base
                ob2 = ob_state["t"]
                if tq >= 24 and tq % 2 == 1:
                    nc.vector.tensor_copy(ob2[:rows, tq - base, :],
                                          ps[:rows, :])
                else:
                    nc.scalar.copy(ob2[:rows, tq - base, :], ps[:rows, :])
                if tq == TQ - 1 or (tq + 1) in bases:
                    pending_stores.append((128 * base,
                                           128 * (tq - base) + rows, ob2))

            # 5-stage pipeline: S(j) | sm_a(j-1) | sm_b(j-2) | slack | bk(j-4)
            its = [(wg2, c) for wg2 in range(WG2) for c in range(CH)]
            NIT = len(its)
            stage_s, stage_p, stage_t = {}, {}, {}
            state = {"done": 0, "ready": 0}

            def op_some(nmax):
                while state["done"] < state["ready"] and nmax > 0:
                    outproj(state["done"])
                    state["done"] += 1
                    nmax -= 1

            ready_updates = []
            for j in range(NIT + 4):
                # outproj first: its PSUM is drained early in the body so the
                # ACT drain never gates this body's exp chain.  Tiles become
                # eligible two bodies after their supergroup's last BK so the
                # attnT drains are never chased.
                flush_stores()
                for (eb, rv) in list(ready_updates):
                    if j >= eb:
                        state["ready"] = max(state["ready"], rv)
                        ready_updates.remove((eb, rv))
                op_some(2 if j < NIT else 3)
                if j < NIT:
                    stage_s[j] = s_stage(*its[j])
                    stage_p[j] = sm_a(stage_s.pop(j), use_dve=(j >= NIT - 2))
                if j == 3:
                    preamble()
                if 0 <= j - 2 < NIT:
                    stage_t[j - 2] = sm_b(stage_p.pop(j - 2))
                if 0 <= j - 4 < NIT:
                    i = j - 4
                    bit = its[i]
                    bk_stage(bit[0], bit[1], stage_t.pop(i))
                    if bit[1] == CH - 1:
                        rv = TQ if bit[0] == WG2 - 1 else 8 * (bit[0] + 1)
                        ready_updates.append((j + 1, rv))
            state["ready"] = TQ
            op_some(TQ)
            flush_stores()


def build(T=T_FULL):
    nc = bacc.Bacc("TRN2", target_bir_lowering=False, debug=False,
                   num_devices=N_CORES)
    x_d = nc.dram_tensor("x", [T, D], F32, kind="ExternalInput")
    wqkv_d = nc.dram_tensor("w_qkv", [D, 3 * D], F32, kind="ExternalInput")
    wout_d = nc.dram_tensor("w_out", [D, D], F32, kind="ExternalInput")
    out_d = nc.dram_tensor("out", [T, D], F32, kind="ExternalOutput")
    with tile.TileContext(nc) as tc:
        _emit(nc, tc, x_d.ap(), wqkv_d.ap(), wout_d.ap(), out_d.ap(), T)
    nc.compile()
    return nc


_NC_CACHE = {}


def kernel(x, w_qkv, w_out):
    x = np.ascontiguousarray(np.asarray(x, dtype=np.float32))
    w_qkv = np.ascontiguousarray(np.asarray(w_qkv, dtype=np.float32))
    w_out = np.ascontiguousarray(np.asarray(w_out, dtype=np.float32))
    assert x.shape == (B, T_FULL, D)

    if "nc" not in _NC_CACHE:
        _NC_CACHE["nc"] = build(T_FULL)
    nc = _NC_CACHE["nc"]

    in_maps = [
        {"x": x[b], "w_qkv": w_qkv, "w_out": w_out} for b in range(N_CORES)
    ]
    last_err = None
    for _attempt in range(4):
        try:
            res = run_bass_kernel_spmd(nc, in_maps, core_ids=list(range(N_CORES)))
            break
        except Exception as e:  # transient NRT device errors
            last_err = e
            try:  # force a fresh PJRT client before retrying
                import jax
                jax.clear_caches()
                jax.extend.backend.clear_backends()
            except Exception:
                pass
            import time as _time
            _time.sleep(5)
    else:
        raise last_err
    return np.stack([res.results[b]["out"] for b in range(N_CORES)], axis=0)
